# revision 66
# baseline (speedup 1.0000x reference)
"""Trainium2 Bass kernel for nn_DetectionLoss (YOLO-style detection loss).

Strategy (pure data parallel over 8 NeuronCores, 256 images each):
  - Host relayouts det to row-per-cell [img*169+cell, 128ch] bf16 (125 ch
    + 3 zero pad). The object gather is then two GPSIMD dma_gather calls
    (4096 indexed 256B-row fetches each) whose output lands DIRECTLY in
    the object-major [p=(b%4)*32+o, j2, ch] layout - no on-chip
    transposes, and the device reads only ~3 MB instead of 22 MB.
  - Gather indices (img*169 + cell of each object) are computed on-chip
    from gt_boxes (floor/k chain + PE partition-shuffle into the wrapped
    idx layout).
  - DVE does IoU / argmax / last-writer-wins dedup / loss assembly in 2
    passes (one per gather half); dedup runs image-per-partition
    ([128, 32*32] ops); class terms run in bf16 2x mode.
  - Dense no-obj conf sum reads a host-extracted [5, 43264] f32 copy of
    the conf channels (contiguous rows).
  - Output: per-core partial sums [128, 16]; host reduces across cores.
"""
import numpy as np

GRID = 13
NA = 5
NCLS = 20
CH = 25
NCH = NA * CH          # 125
CELLS = GRID * GRID    # 169
O = 32                 # objects per image
B = 2048               # global batch
NCORES = 8
BLOC = B // NCORES     # 256 images per core
ROW = BLOC * CELLS     # 43264 cells per core
NOBJ = BLOC * O        # 8192 objects per core
J2 = NOBJ // 128       # 64 object columns
NPASS = 2              # one pass per gather half (128 images each)
JPP = J2 // NPASS      # 32

ANCHORS = np.array([1.3221, 1.73145, 3.19275, 4.00944, 5.05587,
                    8.09892, 9.47112, 4.84053, 11.2364, 10.0071],
                   dtype=np.float32)

_CACHE = {}


def _make_consts():
    """Host-precomputed, data-independent constant input tensors."""
    consts = {}
    consts["c_ident"] = np.eye(128, dtype=np.float32)
    consts["c_iota5"] = np.tile(np.arange(5, dtype=np.float32), (128, 1))
    consts["c_iota5m"] = np.tile(np.arange(5, dtype=np.float32) - 99.0, (128, 1))
    consts["c_iota20"] = np.tile(np.arange(NCLS, dtype=np.float32), (128, 1))
    # anchor w/h prescale for permuted columns 10..20 = (r=2|3, a): [s2*5, s3*5]
    s23 = np.concatenate([ANCHORS[0::2] / GRID, ANCHORS[1::2] / GRID]) \
        .astype(np.float32)
    consts["c_s23"] = np.tile(s23, (128, 1))
    # strict upper-triangular pair mask over (o, o2): 1.0 iff o2 > o
    tri = (np.arange(O)[None, :] > np.arange(O)[:, None]).astype(np.float32)
    consts["c_tri"] = np.tile(tri.reshape(1, O * O), (128, 1))
    # rowbase for the wrapped idx layout: rbw[p, h*256+s] = 169*(s//2)
    col = np.arange(512)
    consts["c_rbw"] = np.tile(
        (CELLS * ((col % 256) // 2)).astype(np.float32), (128, 1))
    return consts


def _build():
    """Build the Bass module (emitted once, cached)."""
    import concourse.bacc as bacc
    import concourse.tile as tile
    from concourse import mybir

    f32 = mybir.dt.float32
    bf16 = mybir.dt.bfloat16
    i16 = mybir.dt.int16
    ALU = mybir.AluOpType
    AX = mybir.AxisListType
    ACT = mybir.ActivationFunctionType

    nc = bacc.Bacc(None, target_bir_lowering=False, debug=False)

    detg = nc.dram_tensor("detg", [ROW, 128], bf16, kind="ExternalInput")
    conf = nc.dram_tensor("conf", [NA, 128, ROW // 128], f32,
                          kind="ExternalInput")
    gtb = nc.dram_tensor("gtb", [128, J2 * 4], f32, kind="ExternalInput")
    gtw = nc.dram_tensor("gtw", [128, 1024], f32, kind="ExternalInput")
    clsf = nc.dram_tensor("clsf", [128, J2], f32, kind="ExternalInput")
    c_ident = nc.dram_tensor("c_ident", [128, 128], f32, kind="ExternalInput")
    c_iota5 = nc.dram_tensor("c_iota5", [128, 5], f32, kind="ExternalInput")
    c_iota5m = nc.dram_tensor("c_iota5m", [128, 5], f32, kind="ExternalInput")
    c_iota20 = nc.dram_tensor("c_iota20", [128, NCLS], f32,
                              kind="ExternalInput")
    c_s23 = nc.dram_tensor("c_s23", [128, 2 * NA], f32, kind="ExternalInput")
    c_tri = nc.dram_tensor("c_tri", [128, O * O], f32, kind="ExternalInput")
    c_rbw = nc.dram_tensor("c_rbw", [128, 512], f32, kind="ExternalInput")
    out = nc.dram_tensor("out", [128, 32], f32, kind="ExternalOutput")

    with tile.TileContext(nc) as tc:
        with tc.tile_pool(name="cpool", bufs=1) as cp, \
             tc.tile_pool(name="work", bufs=1) as wk, \
             tc.tile_pool(name="psA", bufs=2, space="PSUM") as psA:

            # ---- small loads (gtw + rowbase first: idx critical path) ----
            t_gtw = wk.tile([128, 1024], f32)
            nc.sync.dma_start(t_gtw[:], gtw[:])
            t_rbw = cp.tile([128, 512], f32)
            nc.sync.dma_start(t_rbw[:], c_rbw[:])
            t_gtb = wk.tile([128, J2 * 4], f32)
            nc.sync.dma_start(t_gtb[:], gtb[:])
            t_cls = wk.tile([128, J2], f32)
            nc.sync.dma_start(t_cls[:], clsf[:])
            t_id = cp.tile([128, 128], f32)
            t_i5 = cp.tile([128, 5], f32)
            t_i5m = cp.tile([128, 5], f32)
            t_i20 = cp.tile([128, NCLS], f32)
            t_s23 = cp.tile([128, 2 * NA], f32)
            t_tri = cp.tile([128, O * O], f32)
            nc.sync.dma_start(t_id[:], c_ident[:])
            nc.sync.dma_start(t_i5[:], c_iota5[:])
            nc.sync.dma_start(t_i5m[:], c_iota5m[:])
            nc.sync.dma_start(t_i20[:], c_iota20[:])
            nc.sync.dma_start(t_s23[:], c_s23[:])
            nc.sync.dma_start(t_tri[:], c_tri[:])

            # ---- gather row indices, computed directly in the wrapped
            # layout from the host-wrapped x/y copies ----
            t_wfx = wk.tile([128, 512], f32)
            t_wfy = wk.tile([128, 512], f32)
            t_wgx = wk.tile([128, 512], f32)
            t_wgy = wk.tile([128, 512], f32)
            t_wi = wk.tile([128, 512], mybir.dt.int32)
            nc.vector.tensor_scalar_mul(t_wfx[:], t_gtw[:, 0:512],
                                        float(GRID))
            nc.vector.tensor_scalar_mul(t_wfy[:], t_gtw[:, 512:1024],
                                        float(GRID))
            for t_f, t_g in ((t_wfx, t_wgx), (t_wfy, t_wgy)):
                nc.vector.tensor_copy(t_wi[:], t_f[:])
                nc.vector.tensor_copy(t_g[:], t_wi[:])
                nc.vector.tensor_tensor(t_wi[:].bitcast(f32), t_g[:], t_f[:],
                                        ALU.is_gt)
                nc.vector.tensor_sub(t_g[:], t_g[:], t_wi[:].bitcast(f32))
            nc.vector.scalar_tensor_tensor(
                out=t_wfx[:], in0=t_wgy[:], scalar=float(GRID), in1=t_wgx[:],
                op0=ALU.mult, op1=ALU.add)
            nc.vector.tensor_add(t_wfx[:], t_wfx[:], t_rbw[:])
            # 2 idx tiles (dma_gather idx APs must stay < 512B offset)
            t_iw = []
            for h in range(2):
                t_i = wk.tile([128, 256], i16, name=f"t_iw{h}")
                t_iw.append(t_i)
                nc.scalar.activation(t_i[:], t_wfx[:, h * 256:(h + 1) * 256],
                                     ACT.Copy)

            gv = t_gtb[:].rearrange("p (j c) -> p j c", c=4)
            x_ap = gv[:, :, 0]
            y_ap = gv[:, :, 1]
            w_ap = gv[:, :, 2]
            h_ap = gv[:, :, 3]

            # ---- cell coords (DVE, object-major [128, 64]) ----
            t_mx = wk.tile([128, J2], f32)
            t_my = wk.tile([128, J2], f32)
            t_tx = wk.tile([128, J2], f32)
            t_ty = wk.tile([128, J2], f32)
            t_gx = wk.tile([128, J2], f32)
            t_gy = wk.tile([128, J2], f32)
            t_k = wk.tile([128, J2], f32)
            t_scr0 = wk.tile([128, J2], f32)
            nc.vector.tensor_scalar_mul(t_mx[:], x_ap, float(GRID))
            nc.vector.tensor_scalar_mul(t_my[:], y_ap, float(GRID))
            # floor(v), robust to the fp->int rounding mode:
            #   i = cvt(v); fi = cvt_back(i); gx = fi - (fi > v)
            t_i32 = wk.tile([128, J2], mybir.dt.int32)
            for t_m_, t_g_ in ((t_mx, t_gx), (t_my, t_gy)):
                nc.vector.tensor_copy(t_i32[:], t_m_[:])
                nc.vector.tensor_copy(t_g_[:], t_i32[:])
                nc.vector.tensor_tensor(t_scr0[:], t_g_[:], t_m_[:], ALU.is_gt)
                nc.vector.tensor_sub(t_g_[:], t_g_[:], t_scr0[:])
            nc.vector.tensor_sub(t_tx[:], t_mx[:], t_gx[:])
            nc.vector.tensor_sub(t_ty[:], t_my[:], t_gy[:])
            nc.vector.scalar_tensor_tensor(
                out=t_k[:], in0=t_gy[:], scalar=float(GRID), in1=t_gx[:],
                op0=ALU.mult, op1=ALU.add)

            # ---- gt-side quantities hoisted out of the passes ----
            t_hw2 = wk.tile([128, J2], f32)
            t_hh2 = wk.tile([128, J2], f32)
            t_gx0 = wk.tile([128, J2], f32)
            t_gy0 = wk.tile([128, J2], f32)
            t_gx1 = wk.tile([128, J2], f32)
            t_gy1 = wk.tile([128, J2], f32)
            t_a1 = wk.tile([128, J2], f32)
            t_mm2 = wk.tile([128, J2], f32)
            nc.vector.tensor_scalar_mul(t_hw2[:], w_ap, 0.5)
            nc.vector.tensor_scalar_mul(t_hh2[:], h_ap, 0.5)
            nc.vector.tensor_sub(t_gx0[:], x_ap, t_hw2[:])
            nc.vector.tensor_add(t_gx1[:], x_ap, t_hw2[:])
            nc.vector.tensor_sub(t_gy0[:], y_ap, t_hh2[:])
            nc.vector.tensor_add(t_gy1[:], y_ap, t_hh2[:])
            # +1 folded in for the intersection-width computation
            nc.vector.tensor_scalar_add(t_gx1[:], t_gx1[:], 1.0)
            nc.vector.tensor_scalar_add(t_gy1[:], t_gy1[:], 1.0)
            nc.vector.tensor_scalar(t_a1[:], t_hw2[:], 2.0, 1.0,
                                    ALU.mult, ALU.add)
            nc.vector.tensor_scalar(t_mm2[:], t_hh2[:], 2.0, 1.0,
                                    ALU.mult, ALU.add)
            nc.vector.tensor_mul(t_a1[:], t_a1[:], t_mm2[:])
            # coord targets (tx, ty, w, h) interleaved, bf16
            t_txyz = wk.tile([128, J2 * 4], bf16)
            tv = t_txyz[:].rearrange("p (j c) -> p j c", c=4)
            nc.vector.tensor_copy(tv[:, :, 0], t_tx[:])
            nc.vector.tensor_copy(tv[:, :, 1], t_ty[:])
            nc.vector.tensor_copy(tv[:, :, 2], w_ap)
            nc.vector.tensor_copy(tv[:, :, 3], h_ap)

            # ---- the object gather: dma_gather (1024 objects each) ----
            # out[p, j2l, ch] = detg[idx[n = j2l*128 + p], :] with
            # n = (b%128)*32 + o -> p = (b%4)*32 + o, j2l = (b%128)//4
            # HW quirks: num_idxs >= 2048 crashes; idx AP byte offsets
            # must stay < 512 (two [128, 256] tiles -> offsets 0..384)
            t_GTq = []
            for q in range(NPASS):
                t_GT = wk.tile([128, JPP * 128], bf16, name=f"t_GT{q}")
                t_GTq.append(t_GT)
            for g in range(8):
                q, gq = g // 4, g % 4
                nc.gpsimd.dma_gather(
                    out_ap=t_GTq[q][:, gq * 1024:(gq + 1) * 1024]
                    .rearrange("p (j ch) -> p j ch", ch=128),
                    in_ap=detg[(g // 4) * (ROW // 2):(g // 4 + 1) * (ROW // 2)],
                    idxs_ap=t_iw[g // 4][:, (g % 4) * 64:(g % 4 + 1) * 64],
                    num_idxs=1024, num_idxs_reg=1024, elem_size=128)

            t_stage = wk.tile([128, 32], f32)
            nc.vector.memset(t_stage[:], 0.0)

            # ---- dense conf sum (independent of gathers; runs early) ----
            for a in range(NA):
                t_cf = wk.tile([128, ROW // 128], f32, tag="cf", bufs=2,
                               name=f"t_cf{a}")
                nc.sync.dma_start(t_cf[:], conf[a])
                t_cfs = wk.tile([128, ROW // 128], f32, tag="cfs", bufs=2,
                                name=f"t_cfs{a}")
                nc.vector.tensor_mul(t_cfs[:], t_cf[:], t_cf[:])
                nc.vector.reduce_sum(
                    t_stage[:, 20 + a:21 + a], t_cfs[:], axis=AX.X)

            # per-pass work tiles (reused across passes)
            def w5(nm):
                return wk.tile([128, JPP * NA], f32, name=nm)
            t_iou = w5("t_iou"); t_scr = w5("t_scr"); t_scr2 = w5("t_scr2")
            t_bx0 = w5("t_bx0"); t_by0 = w5("t_by0")
            t_bx1 = w5("t_bx1"); t_by1 = w5("t_by1")
            t_ix0 = w5("t_ix0"); t_iy0 = w5("t_iy0")
            t_inter = w5("t_inter"); t_den = w5("t_den")
            t_ohA = w5("t_ohA")

            def w1(nm):
                return wk.tile([128, JPP], f32, name=nm)
            t_mm = w1("t_mm")
            t_aidx = w1("t_aidx"); t_sid = w1("t_sid"); t_win = w1("t_win")
            t_sidm = wk.tile([128, O], f32)       # [p=bi*32+j2l, o]
            t_deadm = wk.tile([128, O], f32)      # [p=bi*32+j2l, o]
            nc.vector.memset(t_sidm[:], 0.0)
            t_eqp = wk.tile([128, O * O], f32)    # [p=(bi,j2l), o*o2]
            # class onehot for ALL j2 (pass-independent; runs pre-gather)
            t_oh = wk.tile([128, J2 * NCLS], bf16)
            nc.vector.tensor_tensor(
                t_oh[:].rearrange("p (j c) -> p j c", c=NCLS),
                t_cls[:].rearrange("p (j one) -> p j one", one=1)
                .to_broadcast([128, J2, NCLS]),
                t_i20[:].rearrange("p (one c) -> p one c", one=1)
                .to_broadcast([128, J2, NCLS]),
                ALU.is_equal)
            t_qc = wk.tile([128, JPP * NA * NCLS], bf16)
            t_half = wk.tile([128, JPP * NA * 10], bf16)
            t_qcls = wk.tile([128, JPP * NA], bf16)
            t_s2sb = wk.tile([128, JPP * NA], bf16)
            t_dv = wk.tile([128, JPP * NA * 4], bf16)
            # stacked masked terms: [p, term(5), j, a]
            t_terms = wk.tile([128, 5 * JPP * NA], f32)
            t_wm = wk.tile([128, 5 * JPP * NA], f32)

            def r5(t):
                return t[:].rearrange("p (j a) -> p j a", a=NA)

            def trm(i):  # term slice i as [p, j, a]
                return t_terms[:, i * JPP * NA:(i + 1) * JPP * NA] \
                    .rearrange("p (j a) -> p j a", a=NA)

            for ps in range(NPASS):
                t_GT = t_GTq[ps]
                t_GSQ = wk.tile([128, JPP * NA * NCLS], bf16,
                                name=f"t_GSQ{ps}", tag="gsq", bufs=2)
                gfull = t_GT[:].rearrange("p (j ch) -> p j ch", ch=128)
                # prescale columns 10..20 (= q2, q3 across anchors) in place
                nc.vector.tensor_tensor(
                    gfull[:, :, 10:20], gfull[:, :, 10:20],
                    t_s23[:].rearrange("p (one s) -> p one s", one=1)
                    .to_broadcast([128, JPP, 10]),
                    ALU.mult)
                # class squares only (conf squares fused into ACT ops below)
                nc.scalar.activation(
                    t_GSQ[:].rearrange("p (j c) -> p j c", c=NA * NCLS),
                    gfull[:, :, 25:125], ACT.Square)

                jsl = slice(ps * JPP, (ps + 1) * JPP)
                q0 = gfull[:, :, 0:5]
                q1 = gfull[:, :, 5:10]
                q2 = gfull[:, :, 10:15]   # pre-scaled by anchor w / GRID
                q3 = gfull[:, :, 15:20]   # pre-scaled by anchor h / GRID
                q4 = gfull[:, :, 20:25]
                qclsv = gfull[:, :, 25:125].rearrange(
                    "p j (a c) -> p j a c", c=NCLS)   # [p, j, a, 20] bf16
                qclssq = t_GSQ[:].rearrange(
                    "p (j a c) -> p j a c", a=NA, c=NCLS)

                def b5(ap2d):  # [128, JPP] -> broadcast [128, JPP, 5]
                    return ap2d.rearrange("p (j one) -> p j one", one=1) \
                               .to_broadcast([128, JPP, NA])

                def c5(tile1):  # const [128, 5] -> [128, JPP, 5]
                    return tile1[:].rearrange("p (one a) -> p one a", one=1) \
                                   .to_broadcast([128, JPP, NA])

                # ---- IoU (per object x anchor) ----
                # bx0 = (px+gx)/13 - pw/2 ; by0 = (py+gy)/13 - ph/2
                nc.vector.tensor_tensor(r5(t_bx0), q0, b5(t_gx[:, jsl]),
                                        ALU.add)
                nc.vector.tensor_scalar_mul(t_bx0[:], t_bx0[:], 1.0 / GRID)
                nc.vector.scalar_tensor_tensor(
                    out=r5(t_bx0), in0=q2, scalar=-0.5, in1=r5(t_bx0),
                    op0=ALU.mult, op1=ALU.add)
                nc.vector.tensor_tensor(r5(t_by0), q1, b5(t_gy[:, jsl]),
                                        ALU.add)
                nc.vector.tensor_scalar_mul(t_by0[:], t_by0[:], 1.0 / GRID)
                nc.vector.scalar_tensor_tensor(
                    out=r5(t_by0), in0=q3, scalar=-0.5, in1=r5(t_by0),
                    op0=ALU.mult, op1=ALU.add)
                # pw+1, ph+1 (used for both bx1+1/by1+1 and a2); the gt-side
                # +1s are folded into the hoisted gx1/gy1
                nc.vector.tensor_scalar(r5(t_scr), q2, 1.0, 1.0,
                                        ALU.mult, ALU.add)
                nc.vector.tensor_scalar(r5(t_scr2), q3, 1.0, 1.0,
                                        ALU.mult, ALU.add)
                nc.vector.tensor_tensor(r5(t_bx1), r5(t_bx0), r5(t_scr),
                                        ALU.add)
                nc.vector.tensor_tensor(r5(t_by1), r5(t_by0), r5(t_scr2),
                                        ALU.add)
                # intersection (+1 folded): ix1+1 = min(bx1+1, gx1+1)
                nc.vector.tensor_tensor(r5(t_ix0), r5(t_bx0), b5(t_gx0[:, jsl]),
                                        ALU.max)
                nc.vector.tensor_tensor(r5(t_iy0), r5(t_by0), b5(t_gy0[:, jsl]),
                                        ALU.max)
                nc.vector.tensor_tensor(r5(t_bx1), r5(t_bx1), b5(t_gx1[:, jsl]),
                                        ALU.min)
                nc.vector.tensor_tensor(r5(t_by1), r5(t_by1), b5(t_gy1[:, jsl]),
                                        ALU.min)
                nc.vector.tensor_sub(t_bx1[:], t_bx1[:], t_ix0[:])
                nc.vector.tensor_sub(t_by1[:], t_by1[:], t_iy0[:])
                nc.vector.tensor_mul(t_inter[:], t_bx1[:], t_by1[:])
                # a2 = (pw+1)*(ph+1); denom = a1 + a2 - inter
                nc.vector.tensor_mul(t_den[:], t_scr[:], t_scr2[:])
                nc.vector.tensor_tensor(r5(t_den), r5(t_den), b5(t_a1[:, jsl]),
                                        ALU.add)
                nc.vector.tensor_sub(t_den[:], t_den[:], t_inter[:])
                nc.vector.reciprocal(t_den[:], t_den[:])
                nc.vector.tensor_mul(t_iou[:], t_inter[:], t_den[:])

                # ---- argmax over anchors (first max wins) ----
                nc.vector.reduce_max(t_mm[:], r5(t_iou), axis=AX.X)
                nc.vector.tensor_tensor(
                    r5(t_scr), r5(t_iou), b5(t_mm), ALU.is_equal)
                nc.vector.tensor_tensor(
                    r5(t_scr2), r5(t_scr), c5(t_i5m), ALU.mult)
                nc.vector.tensor_reduce(
                    t_aidx[:], r5(t_scr2), axis=AX.X, op=ALU.min)
                nc.vector.tensor_scalar_add(t_aidx[:], t_aidx[:], 99.0)

                # ---- slot id s = 169*aidx + k ; last-writer-wins dedup ----
                # layout: object (b,o) at partition p=(bi=b%4)*32+o, col j2=b//4
                nc.vector.scalar_tensor_tensor(
                    out=t_sid[:], in0=t_aidx[:], scalar=float(CELLS),
                    in1=t_k[:, jsl], op0=ALU.mult, op1=ALU.add)
                # transpose -> [j2l, (bi,o)], then 4 ACT copies pack an
                # image per partition: sidm[p = bi*32 + j2l, o] (partition
                # starts must be 32-aligned, so 16 rows used per 32-block).
                # The coord/conf/class DVE blocks below are emitted between
                # the PE/ACT legs so their latency hides under DVE compute.
                t_tp1 = psA.tile([JPP, 128], f32, space="PSUM", tag="ded",
                                 bufs=1)
                nc.tensor.transpose(out=t_tp1[:], in_=t_sid[:],
                                    identity=t_id[:])
                for bi in range(4):
                    nc.scalar.activation(
                        t_sidm[bi * 32:bi * 32 + JPP, :],
                        t_tp1[:, bi * O:(bi + 1) * O], ACT.Copy)

                # ---- coord SSE (-> term slice 0) ----
                # iterate (j, c, a) so the bf16 reads stay stride-1;
                # t_dv is stored [p, j, c, a]
                dvv = t_dv[:].rearrange("p (j c a) -> p j c a", c=4, a=NA)
                nc.vector.tensor_tensor(
                    dvv,
                    gfull[:, :, 0:20].rearrange("p j (c a) -> p j c a", a=NA),
                    t_txyz[:, ps * JPP * 4:(ps + 1) * JPP * 4]
                    .rearrange("p (j c one) -> p j c one", one=1, c=4)
                    .to_broadcast([128, JPP, 4, NA]),
                    ALU.subtract)
                nc.vector.tensor_mul(t_dv[:], t_dv[:], t_dv[:])
                # sum over c with 3 contiguous adds (avoids a strided reduce)
                nc.vector.tensor_tensor(
                    trm(0), dvv[:, :, 0, :], dvv[:, :, 1, :], ALU.add)
                nc.vector.tensor_tensor(
                    r5(t_scr2), dvv[:, :, 2, :], dvv[:, :, 3, :], ALU.add)
                nc.vector.tensor_tensor(
                    trm(0), trm(0), r5(t_scr2), ALU.add)

                # ---- conf terms: (1-q4)^2 -> slice 1, q4^2 -> slice 2 ----
                nc.vector.tensor_scalar(
                    r5(t_scr), q4, -1.0, 1.0, ALU.mult, ALU.add)
                nc.vector.tensor_mul(trm(1), r5(t_scr), r5(t_scr))
                nc.vector.tensor_tensor(trm(2), q4, q4, ALU.mult)

                # ---- class terms (bf16 2x) -> slice 3 ----
                qcv = t_qc[:].rearrange("p (j a c) -> p j a c", a=NA, c=NCLS)
                nc.vector.tensor_tensor(
                    qcv, qclsv,
                    t_oh[:, ps * JPP * NCLS:(ps + 1) * JPP * NCLS]
                    .rearrange("p (j one c) -> p j one c", one=1, c=NCLS)
                    .to_broadcast([128, JPP, NA, NCLS]),
                    ALU.mult)
                # halve the 20-wide reduces with one bf16-2x add first
                hv = t_half[:].rearrange("p (j a c) -> p j a c", a=NA, c=10)
                qcv4 = t_qc[:].rearrange("p (j a h c) -> p j a h c",
                                         a=NA, h=2, c=10)
                sqv4 = t_GSQ[:].rearrange("p (j a h c) -> p j a h c",
                                          a=NA, h=2, c=10)
                with nc.allow_low_precision(reason="20-elem sums, fp32 "
                                            "internal, 2e-2 tolerance"):
                    nc.vector.tensor_tensor(
                        hv, qcv4[:, :, :, 0, :], qcv4[:, :, :, 1, :], ALU.add)
                    nc.vector.tensor_reduce(
                        r5(t_qcls), hv, axis=AX.X, op=ALU.add)
                    nc.vector.tensor_tensor(
                        hv, sqv4[:, :, :, 0, :], sqv4[:, :, :, 1, :], ALU.add)
                    nc.vector.tensor_reduce(
                        r5(t_s2sb), hv, axis=AX.X, op=ALU.add)
                # cls_t = S2 - 2*qcls  (the +1 handled via sum(W))
                nc.vector.scalar_tensor_tensor(
                    out=trm(3), in0=r5(t_qcls), scalar=-2.0, in1=r5(t_s2sb),
                    op0=ALU.mult, op1=ALU.add)

                # ---- dedup pairwise compare (image per partition) ----
                sma = t_sidm[:].rearrange("p (o one) -> p o one", one=1) \
                               .to_broadcast([128, O, O])
                smb = t_sidm[:].rearrange("p (one o2) -> p one o2", one=1) \
                               .to_broadcast([128, O, O])
                eqv = t_eqp[:].rearrange("p (o o2) -> p o o2", o2=O)
                nc.vector.tensor_tensor(eqv, sma, smb, ALU.is_equal)
                triv = t_tri[:].rearrange("p (o o2) -> p o o2", o2=O)
                nc.vector.tensor_tensor(eqv, eqv, triv, ALU.mult)
                nc.vector.tensor_reduce(
                    t_deadm[:].rearrange("p (o one) -> p o one", one=1),
                    eqv, axis=AX.X, op=ALU.max)
                # transpose back -> [o, (bi*32+j2l)], 4 ACT copies ->
                # win[(bi,o), j2l]
                t_tp2 = psA.tile([O, 128], f32, space="PSUM", tag="ded2",
                                 bufs=1)
                nc.tensor.transpose(out=t_tp2[:], in_=t_deadm[:],
                                    identity=t_id[:])
                # (independent DVE op emitted to cover the transpose latency)
                nc.vector.tensor_tensor(
                    r5(t_ohA), b5(t_aidx), c5(t_i5), ALU.is_equal)
                for bi in range(4):
                    nc.scalar.activation(
                        t_win[bi * O:(bi + 1) * O, :],
                        t_tp2[:, bi * 32:bi * 32 + JPP], ACT.Copy)
                nc.vector.tensor_scalar(
                    t_win[:], t_win[:], -1.0, 1.0, ALU.mult, ALU.add)
                # W = onehot(aidx) * win -> term slice 4
                nc.vector.tensor_tensor(
                    trm(4), r5(t_ohA), b5(t_win), ALU.mult)

                # ---- masked accumulate: 2 ops for all 5 terms ----
                nc.vector.tensor_tensor(
                    t_wm[:].rearrange("p (t ja) -> p t ja", t=5),
                    t_terms[:].rearrange("p (t ja) -> p t ja", t=5),
                    t_terms[:, 4 * JPP * NA:5 * JPP * NA]
                    .rearrange("p (one ja) -> p one ja", one=1)
                    .to_broadcast([128, 5, JPP * NA]),
                    ALU.mult)
                nc.vector.tensor_reduce(
                    t_stage[:, ps * 5:(ps + 1) * 5]
                    .rearrange("p (t one) -> p t one", one=1),
                    t_wm[:].rearrange("p (t ja) -> p t ja", t=5),
                    axis=AX.X, op=ALU.add)

            nc.sync.dma_start(out[:], t_stage[:])

    nc.compile()
    return nc


def _get_built():
    if "nc" not in _CACHE:
        _CACHE["nc"] = _build()
        _CACHE["consts"] = _make_consts()
    return _CACHE["nc"], _CACHE["consts"]


def _prep_inputs(detection_result, gt_boxes, gt_class):
    """Host-side layout marshalling (data-independent reshapes only)."""
    import ml_dtypes
    det = np.asarray(detection_result, dtype=np.float32)
    # row-per-cell bf16: [core][img*169+cell][128ch'], where the channel
    # columns are permuted so every per-quantity view is contiguous:
    # ch' = r*5 + a for coord/conf r<5, then 25 + a*20 + c for classes
    perm = np.empty(NCH, dtype=np.int64)
    for r in range(5):
        for a in range(NA):
            perm[r * 5 + a] = a * CH + r
    for a in range(NA):
        for c in range(NCLS):
            perm[25 + a * NCLS + c] = a * CH + 5 + c
    det_g = np.zeros((B, CELLS, 128), dtype=ml_dtypes.bfloat16)
    det_g[:, :, :NCH] = det.reshape(B, NCH, CELLS)[:, perm].transpose(0, 2, 1)
    det_g = det_g.reshape(NCORES, ROW, 128)
    # dense copy of the 5 conf channels: [core][anchor][img][cell] f32
    conf = np.ascontiguousarray(
        det.reshape(NCORES, BLOC, NA, CH, CELLS)[:, :, :, 4, :]
        .transpose(0, 2, 1, 3)).reshape(NCORES, NA, 128, ROW // 128)
    # object-major gt: partition p=(b%4)*32+o, col j2=b//4
    gtb = np.asarray(gt_boxes, dtype=np.float32) \
        .reshape(NCORES, J2, 4, O, 4).transpose(0, 2, 3, 1, 4) \
        .reshape(NCORES, 128, J2 * 4)
    gtb = np.ascontiguousarray(gtb)
    # wrapped x/y copies for on-chip gather-index computation:
    # value at [16g+q, h*256+s] = coord of object (b = h*128 + s//2,
    # o = 16*(s%2) + q), replicated across the 8 16-partition groups
    gb = np.asarray(gt_boxes, dtype=np.float32).reshape(NCORES, BLOC, O, 4)
    col = np.arange(512)
    b_of = (col // 256) * 128 + (col % 256) // 2          # [512]
    q = np.arange(16)
    o_of = 16 * (col % 2)[None, :] + q[:, None]           # [16, 512]
    gtw = np.empty((NCORES, 128, 1024), dtype=np.float32)
    for c in range(2):
        w16 = gb[:, b_of[None, :], o_of, c]               # [NCORES, 16, 512]
        gtw[:, :, c * 512:(c + 1) * 512] = np.tile(w16, (1, 8, 1))
    clsf = np.asarray(gt_class).astype(np.float32) \
        .reshape(NCORES, J2, 4, O).transpose(0, 2, 3, 1) \
        .reshape(NCORES, 128, J2)
    clsf = np.ascontiguousarray(clsf)
    return det_g, conf, gtb, gtw, clsf


def _reduce_partials(P):
    """P: [ncores, 128, 32] fp32 partials -> the 4 scalar losses."""
    S = P.astype(np.float64).sum(axis=(0, 1))
    T = S[0:20].reshape(4, 5).sum(axis=0)
    coord, confobj, confsub, clsq, wsum = T
    dense = S[20:25].sum()
    obj_loss = 5.0 * coord + confobj
    no_obj_loss = 0.5 * (dense - confsub)
    conf_loss = clsq + wsum
    loss = obj_loss + no_obj_loss + conf_loss
    return (np.float32(loss), np.float32(obj_loss),
            np.float32(no_obj_loss), np.float32(conf_loss))


LAST_RESULT = None


def kernel(detection_result, gt_boxes, gt_class):
    import os
    from concourse.bass_utils import run_bass_kernel_spmd

    nc, consts = _get_built()
    det_g, conf, gtb, gtw, clsf = _prep_inputs(detection_result, gt_boxes,
                                               gt_class)

    in_maps = []
    for c in range(NCORES):
        m = {"detg": det_g[c], "conf": conf[c], "gtb": gtb[c],
             "gtw": gtw[c], "clsf": clsf[c]}
        m.update(consts)
        in_maps.append(m)

    kw = {}
    if os.environ.get("DETLOSS_TRACE"):
        kw["trace"] = True
        td = os.environ.get("DETLOSS_TRACE_DIR")
        if td:
            os.makedirs(td, exist_ok=True)
            kw["tmpdir"] = td
    res = run_bass_kernel_spmd(nc, in_maps, core_ids=list(range(NCORES)), **kw)
    global LAST_RESULT
    LAST_RESULT = res
    P = np.stack([res.results[c]["out"] for c in range(NCORES)])
    return _reduce_partials(P)


# revision 68
# speedup vs baseline: 1.1717x; 1.1717x over previous
"""Trainium2 Bass kernel for nn_DetectionLoss (YOLO-style detection loss).

Strategy (pure data parallel over 8 NeuronCores, 256 images each):
  - Host relayouts det to row-per-cell [img*169+cell, 128ch] bf16 (125 ch
    + 3 zero pad). The object gather is then two GPSIMD dma_gather calls
    (4096 indexed 256B-row fetches each) whose output lands DIRECTLY in
    the object-major [p=(b%4)*32+o, j2, ch] layout - no on-chip
    transposes, and the device reads only ~3 MB instead of 22 MB.
  - Gather indices (img*169 + cell of each object) are computed on-chip
    from gt_boxes (floor/k chain + PE partition-shuffle into the wrapped
    idx layout).
  - DVE does IoU / argmax / last-writer-wins dedup / loss assembly in 2
    passes (one per gather half); dedup runs image-per-partition
    ([128, 32*32] ops); class terms run in bf16 2x mode.
  - Dense no-obj conf sum reads a host-extracted [5, 43264] f32 copy of
    the conf channels (contiguous rows).
  - Output: per-core partial sums [128, 16]; host reduces across cores.
"""
import numpy as np

GRID = 13
NA = 5
NCLS = 20
CH = 25
NCH = NA * CH          # 125
CELLS = GRID * GRID    # 169
O = 32                 # objects per image
B = 2048               # global batch
NCORES = 8
BLOC = B // NCORES     # 256 images per core
ROW = BLOC * CELLS     # 43264 cells per core
NOBJ = BLOC * O        # 8192 objects per core
J2 = NOBJ // 128       # 64 object columns
NPASS = 2              # one pass per gather half (128 images each)
JPP = J2 // NPASS      # 32

ANCHORS = np.array([1.3221, 1.73145, 3.19275, 4.00944, 5.05587,
                    8.09892, 9.47112, 4.84053, 11.2364, 10.0071],
                   dtype=np.float32)

_CACHE = {}


def _make_consts():
    """Host-precomputed, data-independent constant input tensors."""
    consts = {}
    consts["c_ident"] = np.eye(128, dtype=np.float32)
    consts["c_iota5"] = np.tile(np.arange(5, dtype=np.float32), (128, 1))
    consts["c_iota5m"] = np.tile(np.arange(5, dtype=np.float32) - 99.0, (128, 1))
    consts["c_iota20"] = np.tile(np.arange(NCLS, dtype=np.float32), (128, 1))
    # anchor w/h prescale for permuted columns 10..20 = (r=2|3, a): [s2*5, s3*5]
    s23 = np.concatenate([ANCHORS[0::2] / GRID, ANCHORS[1::2] / GRID]) \
        .astype(np.float32)
    consts["c_s23"] = np.tile(s23, (128, 1))
    # strict upper-triangular pair mask over (o, o2): 1.0 iff o2 > o
    tri = (np.arange(O)[None, :] > np.arange(O)[:, None]).astype(np.float32)
    consts["c_tri"] = np.tile(tri.reshape(1, O * O), (128, 1))
    # rowbase for the wrapped idx layout: rbw[p, h*256+s] = 169*(s//2)
    col = np.arange(512)
    consts["c_rbw"] = np.tile(
        (CELLS * ((col % 256) // 2)).astype(np.float32), (128, 1))
    return consts


def _build():
    """Build the Bass module (emitted once, cached)."""
    import concourse.bacc as bacc
    import concourse.tile as tile
    from concourse import mybir

    f32 = mybir.dt.float32
    bf16 = mybir.dt.bfloat16
    i16 = mybir.dt.int16
    ALU = mybir.AluOpType
    AX = mybir.AxisListType
    ACT = mybir.ActivationFunctionType

    nc = bacc.Bacc(None, target_bir_lowering=False, debug=False)

    detg = nc.dram_tensor("detg", [ROW, 128], bf16, kind="ExternalInput")
    conf = nc.dram_tensor("conf", [NA, 128, ROW // 128], f32,
                          kind="ExternalInput")
    gtb = nc.dram_tensor("gtb", [128, J2 * 4], f32, kind="ExternalInput")
    gtw = nc.dram_tensor("gtw", [128, 1024], f32, kind="ExternalInput")
    clsf = nc.dram_tensor("clsf", [128, J2], f32, kind="ExternalInput")
    c_ident = nc.dram_tensor("c_ident", [128, 128], f32, kind="ExternalInput")
    c_iota5 = nc.dram_tensor("c_iota5", [128, 5], f32, kind="ExternalInput")
    c_iota5m = nc.dram_tensor("c_iota5m", [128, 5], f32, kind="ExternalInput")
    c_iota20 = nc.dram_tensor("c_iota20", [128, NCLS], f32,
                              kind="ExternalInput")
    c_s23 = nc.dram_tensor("c_s23", [128, 2 * NA], f32, kind="ExternalInput")
    c_tri = nc.dram_tensor("c_tri", [128, O * O], f32, kind="ExternalInput")
    c_rbw = nc.dram_tensor("c_rbw", [128, 512], f32, kind="ExternalInput")
    out = nc.dram_tensor("out", [128, 32], f32, kind="ExternalOutput")

    with tile.TileContext(nc) as tc:
        with tc.tile_pool(name="cpool", bufs=1) as cp, \
             tc.tile_pool(name="work", bufs=1) as wk, \
             tc.tile_pool(name="psA", bufs=2, space="PSUM") as psA:

            # ---- small loads (gtw + rowbase first: idx critical path) ----
            t_gtw = wk.tile([128, 1024], f32)
            nc.sync.dma_start(t_gtw[:], gtw[:])
            t_rbw = cp.tile([128, 512], f32)
            nc.sync.dma_start(t_rbw[:], c_rbw[:])
            t_gtb = wk.tile([128, J2 * 4], f32)
            nc.sync.dma_start(t_gtb[:], gtb[:])
            t_cls = wk.tile([128, J2], f32)
            nc.sync.dma_start(t_cls[:], clsf[:])
            t_id = cp.tile([128, 128], f32)
            t_i5 = cp.tile([128, 5], f32)
            t_i5m = cp.tile([128, 5], f32)
            t_i20 = cp.tile([128, NCLS], f32)
            t_s23 = cp.tile([128, 2 * NA], f32)
            t_tri = cp.tile([128, O * O], f32)
            nc.sync.dma_start(t_id[:], c_ident[:])
            nc.sync.dma_start(t_i5[:], c_iota5[:])
            nc.sync.dma_start(t_i5m[:], c_iota5m[:])
            nc.sync.dma_start(t_i20[:], c_iota20[:])
            nc.sync.dma_start(t_s23[:], c_s23[:])
            nc.sync.dma_start(t_tri[:], c_tri[:])

            # ---- gather row indices, computed directly in the wrapped
            # layout from the host-wrapped x/y copies ----
            t_wfx = wk.tile([128, 512], f32)
            t_wfy = wk.tile([128, 512], f32)
            t_wgx = wk.tile([128, 512], f32)
            t_wgy = wk.tile([128, 512], f32)
            t_wi = wk.tile([128, 512], mybir.dt.int32)
            nc.vector.tensor_scalar_mul(t_wfx[:], t_gtw[:, 0:512],
                                        float(GRID))
            nc.vector.tensor_scalar_mul(t_wfy[:], t_gtw[:, 512:1024],
                                        float(GRID))
            for t_f, t_g in ((t_wfx, t_wgx), (t_wfy, t_wgy)):
                nc.vector.tensor_copy(t_wi[:], t_f[:])
                nc.vector.tensor_copy(t_g[:], t_wi[:])
                nc.vector.tensor_tensor(t_wi[:].bitcast(f32), t_g[:], t_f[:],
                                        ALU.is_gt)
                nc.vector.tensor_sub(t_g[:], t_g[:], t_wi[:].bitcast(f32))
            nc.vector.scalar_tensor_tensor(
                out=t_wfx[:], in0=t_wgy[:], scalar=float(GRID), in1=t_wgx[:],
                op0=ALU.mult, op1=ALU.add)
            nc.vector.tensor_add(t_wfx[:], t_wfx[:], t_rbw[:])
            # 2 idx tiles (dma_gather idx APs must stay < 512B offset)
            t_iw = []
            for h in range(2):
                t_i = wk.tile([128, 256], i16, name=f"t_iw{h}")
                t_iw.append(t_i)
                nc.scalar.activation(t_i[:], t_wfx[:, h * 256:(h + 1) * 256],
                                     ACT.Copy)

            gv = t_gtb[:].rearrange("p (j c) -> p j c", c=4)
            x_ap = gv[:, :, 0]
            y_ap = gv[:, :, 1]
            w_ap = gv[:, :, 2]
            h_ap = gv[:, :, 3]

            # ---- cell coords (DVE, object-major [128, 64]) ----
            t_mx = wk.tile([128, J2], f32)
            t_my = wk.tile([128, J2], f32)
            t_tx = wk.tile([128, J2], f32)
            t_ty = wk.tile([128, J2], f32)
            t_gx = wk.tile([128, J2], f32)
            t_gy = wk.tile([128, J2], f32)
            t_k = wk.tile([128, J2], f32)
            t_scr0 = wk.tile([128, J2], f32)
            nc.vector.tensor_scalar_mul(t_mx[:], x_ap, float(GRID))
            nc.vector.tensor_scalar_mul(t_my[:], y_ap, float(GRID))
            # floor(v), robust to the fp->int rounding mode:
            #   i = cvt(v); fi = cvt_back(i); gx = fi - (fi > v)
            t_i32 = wk.tile([128, J2], mybir.dt.int32)
            for t_m_, t_g_ in ((t_mx, t_gx), (t_my, t_gy)):
                nc.vector.tensor_copy(t_i32[:], t_m_[:])
                nc.vector.tensor_copy(t_g_[:], t_i32[:])
                nc.vector.tensor_tensor(t_scr0[:], t_g_[:], t_m_[:], ALU.is_gt)
                nc.vector.tensor_sub(t_g_[:], t_g_[:], t_scr0[:])
            nc.vector.tensor_sub(t_tx[:], t_mx[:], t_gx[:])
            nc.vector.tensor_sub(t_ty[:], t_my[:], t_gy[:])
            nc.vector.scalar_tensor_tensor(
                out=t_k[:], in0=t_gy[:], scalar=float(GRID), in1=t_gx[:],
                op0=ALU.mult, op1=ALU.add)

            # ---- gt-side quantities hoisted out of the passes ----
            t_hw2 = wk.tile([128, J2], f32)
            t_hh2 = wk.tile([128, J2], f32)
            t_gx0 = wk.tile([128, J2], f32)
            t_gy0 = wk.tile([128, J2], f32)
            t_gx1 = wk.tile([128, J2], f32)
            t_gy1 = wk.tile([128, J2], f32)
            t_a1 = wk.tile([128, J2], f32)
            t_mm2 = wk.tile([128, J2], f32)
            nc.vector.tensor_scalar_mul(t_hw2[:], w_ap, 0.5)
            nc.vector.tensor_scalar_mul(t_hh2[:], h_ap, 0.5)
            nc.vector.tensor_sub(t_gx0[:], x_ap, t_hw2[:])
            nc.vector.tensor_add(t_gx1[:], x_ap, t_hw2[:])
            nc.vector.tensor_sub(t_gy0[:], y_ap, t_hh2[:])
            nc.vector.tensor_add(t_gy1[:], y_ap, t_hh2[:])
            # +1 folded in for the intersection-width computation
            nc.vector.tensor_scalar_add(t_gx1[:], t_gx1[:], 1.0)
            nc.vector.tensor_scalar_add(t_gy1[:], t_gy1[:], 1.0)
            nc.vector.tensor_scalar(t_a1[:], t_hw2[:], 2.0, 1.0,
                                    ALU.mult, ALU.add)
            nc.vector.tensor_scalar(t_mm2[:], t_hh2[:], 2.0, 1.0,
                                    ALU.mult, ALU.add)
            nc.vector.tensor_mul(t_a1[:], t_a1[:], t_mm2[:])
            # coord targets (tx, ty, w, h) interleaved, bf16
            t_txyz = wk.tile([128, J2 * 4], bf16)
            tv = t_txyz[:].rearrange("p (j c) -> p j c", c=4)
            nc.vector.tensor_copy(tv[:, :, 0], t_tx[:])
            nc.vector.tensor_copy(tv[:, :, 1], t_ty[:])
            nc.vector.tensor_copy(tv[:, :, 2], w_ap)
            nc.vector.tensor_copy(tv[:, :, 3], h_ap)

            # ---- the object gather: dma_gather (1024 objects each) ----
            # out[p, j2l, ch] = detg[idx[n = j2l*128 + p], :] with
            # n = (b%128)*32 + o -> p = (b%4)*32 + o, j2l = (b%128)//4
            # HW quirks: num_idxs >= 2048 crashes; idx AP byte offsets
            # must stay < 512 (two [128, 256] tiles -> offsets 0..384)
            t_GTq = []
            for q in range(NPASS):
                t_GT = wk.tile([128, JPP * 128], bf16, name=f"t_GT{q}")
                t_GTq.append(t_GT)
            for g in range(8):
                q, gq = g // 4, g % 4
                nc.gpsimd.dma_gather(
                    out_ap=t_GTq[q][:, gq * 1024:(gq + 1) * 1024]
                    .rearrange("p (j ch) -> p j ch", ch=128),
                    in_ap=detg[(g // 4) * (ROW // 2):(g // 4 + 1) * (ROW // 2)],
                    idxs_ap=t_iw[g // 4][:, (g % 4) * 64:(g % 4 + 1) * 64],
                    num_idxs=1024, num_idxs_reg=1024, elem_size=128)

            t_stage = wk.tile([128, 32], f32)
            nc.vector.memset(t_stage[:], 0.0)

            # ---- dense conf sum (independent of gathers; runs early) ----
            for a in range(NA):
                t_cf = wk.tile([128, ROW // 128], f32, tag="cf", bufs=2,
                               name=f"t_cf{a}")
                nc.sync.dma_start(t_cf[:], conf[a])
                t_cfs = wk.tile([128, ROW // 128], f32, tag="cfs", bufs=2,
                                name=f"t_cfs{a}")
                nc.vector.tensor_mul(t_cfs[:], t_cf[:], t_cf[:])
                nc.vector.reduce_sum(
                    t_stage[:, 20 + a:21 + a], t_cfs[:], axis=AX.X)

            # per-pass work tiles (reused across passes)
            def w5(nm):
                return wk.tile([128, JPP * NA], f32, name=nm)
            t_iou = w5("t_iou"); t_scr = w5("t_scr"); t_scr2 = w5("t_scr2")
            t_bx0 = w5("t_bx0"); t_by0 = w5("t_by0")
            t_bx1 = w5("t_bx1"); t_by1 = w5("t_by1")
            t_ix0 = w5("t_ix0"); t_iy0 = w5("t_iy0")
            t_inter = w5("t_inter"); t_den = w5("t_den")
            t_ohA = w5("t_ohA")

            def w1(nm):
                return wk.tile([128, JPP], f32, name=nm)
            t_mm = w1("t_mm")
            t_aidx = w1("t_aidx"); t_sid = w1("t_sid"); t_win = w1("t_win")
            t_sidm = wk.tile([128, O], f32)       # [p=bi*32+j2l, o]
            t_deadm = wk.tile([128, O], f32)      # [p=bi*32+j2l, o]
            nc.vector.memset(t_sidm[:], 0.0)
            t_eqp = wk.tile([128, O * O], f32)    # [p=(bi,j2l), o*o2]
            # class onehot for ALL j2 (pass-independent; runs pre-gather)
            t_oh = wk.tile([128, J2 * NCLS], bf16)
            nc.vector.tensor_tensor(
                t_oh[:].rearrange("p (j c) -> p j c", c=NCLS),
                t_cls[:].rearrange("p (j one) -> p j one", one=1)
                .to_broadcast([128, J2, NCLS]),
                t_i20[:].rearrange("p (one c) -> p one c", one=1)
                .to_broadcast([128, J2, NCLS]),
                ALU.is_equal)
            t_qc = wk.tile([128, JPP * NA * NCLS], bf16)
            t_half = wk.tile([128, JPP * NA * 10], bf16)
            t_qcls = wk.tile([128, JPP * NA], bf16)
            t_s2sb = wk.tile([128, JPP * NA], bf16)
            t_dv = wk.tile([128, JPP * NA * 4], bf16)
            # stacked masked terms: [p, term(5), j, a]
            t_terms = wk.tile([128, 5 * JPP * NA], f32)
            t_wm = wk.tile([128, 5 * JPP * NA], f32)

            def r5(t):
                return t[:].rearrange("p (j a) -> p j a", a=NA)

            def trm(i):  # term slice i as [p, j, a]
                return t_terms[:, i * JPP * NA:(i + 1) * JPP * NA] \
                    .rearrange("p (j a) -> p j a", a=NA)

            for ps in range(NPASS):
                t_GT = t_GTq[ps]
                t_GSQ = wk.tile([128, JPP * NA * NCLS], bf16,
                                name=f"t_GSQ{ps}", tag="gsq", bufs=2)
                gfull = t_GT[:].rearrange("p (j ch) -> p j ch", ch=128)
                # prescale columns 10..20 (= q2, q3 across anchors) in place
                nc.vector.tensor_tensor(
                    gfull[:, :, 10:20], gfull[:, :, 10:20],
                    t_s23[:].rearrange("p (one s) -> p one s", one=1)
                    .to_broadcast([128, JPP, 10]),
                    ALU.mult)
                # class squares only (conf squares fused into ACT ops below)
                nc.scalar.activation(
                    t_GSQ[:].rearrange("p (j c) -> p j c", c=NA * NCLS),
                    gfull[:, :, 25:125], ACT.Square)

                jsl = slice(ps * JPP, (ps + 1) * JPP)
                q0 = gfull[:, :, 0:5]
                q1 = gfull[:, :, 5:10]
                q2 = gfull[:, :, 10:15]   # pre-scaled by anchor w / GRID
                q3 = gfull[:, :, 15:20]   # pre-scaled by anchor h / GRID
                q4 = gfull[:, :, 20:25]
                qclsv = gfull[:, :, 25:125].rearrange(
                    "p j (a c) -> p j a c", c=NCLS)   # [p, j, a, 20] bf16
                qclssq = t_GSQ[:].rearrange(
                    "p (j a c) -> p j a c", a=NA, c=NCLS)

                def b5(ap2d):  # [128, JPP] -> broadcast [128, JPP, 5]
                    return ap2d.rearrange("p (j one) -> p j one", one=1) \
                               .to_broadcast([128, JPP, NA])

                def c5(tile1):  # const [128, 5] -> [128, JPP, 5]
                    return tile1[:].rearrange("p (one a) -> p one a", one=1) \
                                   .to_broadcast([128, JPP, NA])

                # ---- IoU (per object x anchor) ----
                # bx0 = (px+gx)/13 - pw/2 ; by0 = (py+gy)/13 - ph/2
                nc.vector.tensor_tensor(r5(t_bx0), q0, b5(t_gx[:, jsl]),
                                        ALU.add)
                nc.vector.tensor_scalar_mul(t_bx0[:], t_bx0[:], 1.0 / GRID)
                nc.vector.scalar_tensor_tensor(
                    out=r5(t_bx0), in0=q2, scalar=-0.5, in1=r5(t_bx0),
                    op0=ALU.mult, op1=ALU.add)
                nc.vector.tensor_tensor(r5(t_by0), q1, b5(t_gy[:, jsl]),
                                        ALU.add)
                nc.vector.tensor_scalar_mul(t_by0[:], t_by0[:], 1.0 / GRID)
                nc.vector.scalar_tensor_tensor(
                    out=r5(t_by0), in0=q3, scalar=-0.5, in1=r5(t_by0),
                    op0=ALU.mult, op1=ALU.add)
                # pw+1, ph+1 (used for both bx1+1/by1+1 and a2); the gt-side
                # +1s are folded into the hoisted gx1/gy1
                nc.vector.tensor_scalar(r5(t_scr), q2, 1.0, 1.0,
                                        ALU.mult, ALU.add)
                nc.vector.tensor_scalar(r5(t_scr2), q3, 1.0, 1.0,
                                        ALU.mult, ALU.add)
                nc.vector.tensor_tensor(r5(t_bx1), r5(t_bx0), r5(t_scr),
                                        ALU.add)
                nc.vector.tensor_tensor(r5(t_by1), r5(t_by0), r5(t_scr2),
                                        ALU.add)
                # intersection (+1 folded): ix1+1 = min(bx1+1, gx1+1)
                nc.vector.tensor_tensor(r5(t_ix0), r5(t_bx0), b5(t_gx0[:, jsl]),
                                        ALU.max)
                nc.vector.tensor_tensor(r5(t_iy0), r5(t_by0), b5(t_gy0[:, jsl]),
                                        ALU.max)
                nc.vector.tensor_tensor(r5(t_bx1), r5(t_bx1), b5(t_gx1[:, jsl]),
                                        ALU.min)
                nc.vector.tensor_tensor(r5(t_by1), r5(t_by1), b5(t_gy1[:, jsl]),
                                        ALU.min)
                nc.vector.tensor_sub(t_bx1[:], t_bx1[:], t_ix0[:])
                nc.vector.tensor_sub(t_by1[:], t_by1[:], t_iy0[:])
                nc.vector.tensor_mul(t_inter[:], t_bx1[:], t_by1[:])
                # a2 = (pw+1)*(ph+1); denom = a1 + a2 - inter
                nc.vector.tensor_mul(t_den[:], t_scr[:], t_scr2[:])
                nc.vector.tensor_tensor(r5(t_den), r5(t_den), b5(t_a1[:, jsl]),
                                        ALU.add)
                nc.vector.tensor_sub(t_den[:], t_den[:], t_inter[:])
                nc.vector.reciprocal(t_den[:], t_den[:])
                nc.vector.tensor_mul(t_iou[:], t_inter[:], t_den[:])

                # ---- argmax over anchors (first max wins) ----
                nc.vector.reduce_max(t_mm[:], r5(t_iou), axis=AX.X)
                nc.vector.tensor_tensor(
                    r5(t_scr), r5(t_iou), b5(t_mm), ALU.is_equal)
                nc.vector.tensor_tensor(
                    r5(t_scr2), r5(t_scr), c5(t_i5m), ALU.mult)
                nc.vector.tensor_reduce(
                    t_aidx[:], r5(t_scr2), axis=AX.X, op=ALU.min)
                nc.vector.tensor_scalar_add(t_aidx[:], t_aidx[:], 99.0)

                # ---- slot id s = 169*aidx + k ; last-writer-wins dedup ----
                # layout: object (b,o) at partition p=(bi=b%4)*32+o, col j2=b//4
                nc.vector.scalar_tensor_tensor(
                    out=t_sid[:], in0=t_aidx[:], scalar=float(CELLS),
                    in1=t_k[:, jsl], op0=ALU.mult, op1=ALU.add)
                # transpose -> [j2l, (bi,o)], then 4 ACT copies pack an
                # image per partition: sidm[p = bi*32 + j2l, o] (partition
                # starts must be 32-aligned, so 16 rows used per 32-block).
                # The coord/conf/class DVE blocks below are emitted between
                # the PE/ACT legs so their latency hides under DVE compute.
                t_tp1 = psA.tile([JPP, 128], f32, space="PSUM", tag="ded",
                                 bufs=1)
                nc.tensor.transpose(out=t_tp1[:], in_=t_sid[:],
                                    identity=t_id[:])
                for bi in range(4):
                    nc.scalar.activation(
                        t_sidm[bi * 32:bi * 32 + JPP, :],
                        t_tp1[:, bi * O:(bi + 1) * O], ACT.Copy)
                sma = t_sidm[:].rearrange("p (o one) -> p o one", one=1) \
                               .to_broadcast([128, O, O])
                smb = t_sidm[:].rearrange("p (one o2) -> p one o2", one=1) \
                               .to_broadcast([128, O, O])
                eqv = t_eqp[:].rearrange("p (o o2) -> p o o2", o2=O)
                nc.vector.tensor_tensor(eqv, sma, smb, ALU.is_equal)
                triv = t_tri[:].rearrange("p (o o2) -> p o o2", o2=O)
                nc.vector.tensor_tensor(eqv, eqv, triv, ALU.mult)
                nc.vector.tensor_reduce(
                    t_deadm[:].rearrange("p (o one) -> p o one", one=1),
                    eqv, axis=AX.X, op=ALU.max)
                t_tp2 = psA.tile([O, 128], f32, space="PSUM", tag="ded2",
                                 bufs=1)
                nc.tensor.transpose(out=t_tp2[:], in_=t_deadm[:],
                                    identity=t_id[:])
                for bi in range(4):
                    nc.scalar.activation(
                        t_win[bi * O:(bi + 1) * O, :],
                        t_tp2[:, bi * 32:bi * 32 + JPP], ACT.Copy)
                nc.vector.tensor_scalar(
                    t_win[:], t_win[:], -1.0, 1.0, ALU.mult, ALU.add)
                nc.vector.tensor_tensor(
                    r5(t_ohA), b5(t_aidx), c5(t_i5), ALU.is_equal)
                nc.vector.tensor_tensor(
                    trm(4), r5(t_ohA), b5(t_win), ALU.mult)

                # ---- coord SSE (-> term slice 0) ----
                # iterate (j, c, a) so the bf16 reads stay stride-1;
                # t_dv is stored [p, j, c, a]
                dvv = t_dv[:].rearrange("p (j c a) -> p j c a", c=4, a=NA)
                nc.vector.tensor_tensor(
                    dvv,
                    gfull[:, :, 0:20].rearrange("p j (c a) -> p j c a", a=NA),
                    t_txyz[:, ps * JPP * 4:(ps + 1) * JPP * 4]
                    .rearrange("p (j c one) -> p j c one", one=1, c=4)
                    .to_broadcast([128, JPP, 4, NA]),
                    ALU.subtract)
                nc.vector.tensor_mul(t_dv[:], t_dv[:], t_dv[:])
                # sum over c with 3 contiguous adds (avoids a strided reduce)
                nc.vector.tensor_tensor(
                    trm(0), dvv[:, :, 0, :], dvv[:, :, 1, :], ALU.add)
                nc.vector.tensor_tensor(
                    r5(t_scr2), dvv[:, :, 2, :], dvv[:, :, 3, :], ALU.add)
                nc.vector.tensor_tensor(
                    trm(0), trm(0), r5(t_scr2), ALU.add)

                # ---- conf terms: (1-q4)^2 -> slice 1, q4^2 -> slice 2 ----
                nc.vector.tensor_scalar(
                    r5(t_scr), q4, -1.0, 1.0, ALU.mult, ALU.add)
                nc.vector.tensor_mul(trm(1), r5(t_scr), r5(t_scr))
                nc.vector.tensor_tensor(trm(2), q4, q4, ALU.mult)

                # ---- class terms (bf16 2x) -> slice 3 ----
                qcv = t_qc[:].rearrange("p (j a c) -> p j a c", a=NA, c=NCLS)
                nc.vector.tensor_tensor(
                    qcv, qclsv,
                    t_oh[:, ps * JPP * NCLS:(ps + 1) * JPP * NCLS]
                    .rearrange("p (j one c) -> p j one c", one=1, c=NCLS)
                    .to_broadcast([128, JPP, NA, NCLS]),
                    ALU.mult)
                # halve the 20-wide reduces with one bf16-2x add first
                hv = t_half[:].rearrange("p (j a c) -> p j a c", a=NA, c=10)
                qcv4 = t_qc[:].rearrange("p (j a h c) -> p j a h c",
                                         a=NA, h=2, c=10)
                sqv4 = t_GSQ[:].rearrange("p (j a h c) -> p j a h c",
                                          a=NA, h=2, c=10)
                with nc.allow_low_precision(reason="20-elem sums, fp32 "
                                            "internal, 2e-2 tolerance"):
                    nc.vector.tensor_tensor(
                        hv, qcv4[:, :, :, 0, :], qcv4[:, :, :, 1, :], ALU.add)
                    nc.vector.tensor_reduce(
                        r5(t_qcls), hv, axis=AX.X, op=ALU.add)
                    nc.vector.tensor_tensor(
                        hv, sqv4[:, :, :, 0, :], sqv4[:, :, :, 1, :], ALU.add)
                    nc.vector.tensor_reduce(
                        r5(t_s2sb), hv, axis=AX.X, op=ALU.add)
                # cls_t = S2 - 2*qcls  (the +1 handled via sum(W))
                nc.vector.scalar_tensor_tensor(
                    out=trm(3), in0=r5(t_qcls), scalar=-2.0, in1=r5(t_s2sb),
                    op0=ALU.mult, op1=ALU.add)

                # ---- masked accumulate: 2 ops for all 5 terms ----
                nc.vector.tensor_tensor(
                    t_wm[:].rearrange("p (t ja) -> p t ja", t=5),
                    t_terms[:].rearrange("p (t ja) -> p t ja", t=5),
                    t_terms[:, 4 * JPP * NA:5 * JPP * NA]
                    .rearrange("p (one ja) -> p one ja", one=1)
                    .to_broadcast([128, 5, JPP * NA]),
                    ALU.mult)
                nc.vector.tensor_reduce(
                    t_stage[:, ps * 5:(ps + 1) * 5]
                    .rearrange("p (t one) -> p t one", one=1),
                    t_wm[:].rearrange("p (t ja) -> p t ja", t=5),
                    axis=AX.X, op=ALU.add)

            nc.sync.dma_start(out[:], t_stage[:])

    nc.compile()
    return nc


def _get_built():
    if "nc" not in _CACHE:
        _CACHE["nc"] = _build()
        _CACHE["consts"] = _make_consts()
    return _CACHE["nc"], _CACHE["consts"]


def _prep_inputs(detection_result, gt_boxes, gt_class):
    """Host-side layout marshalling (data-independent reshapes only)."""
    import ml_dtypes
    det = np.asarray(detection_result, dtype=np.float32)
    # row-per-cell bf16: [core][img*169+cell][128ch'], where the channel
    # columns are permuted so every per-quantity view is contiguous:
    # ch' = r*5 + a for coord/conf r<5, then 25 + a*20 + c for classes
    perm = np.empty(NCH, dtype=np.int64)
    for r in range(5):
        for a in range(NA):
            perm[r * 5 + a] = a * CH + r
    for a in range(NA):
        for c in range(NCLS):
            perm[25 + a * NCLS + c] = a * CH + 5 + c
    det_g = np.zeros((B, CELLS, 128), dtype=ml_dtypes.bfloat16)
    det_g[:, :, :NCH] = det.reshape(B, NCH, CELLS)[:, perm].transpose(0, 2, 1)
    det_g = det_g.reshape(NCORES, ROW, 128)
    # dense copy of the 5 conf channels: [core][anchor][img][cell] f32
    conf = np.ascontiguousarray(
        det.reshape(NCORES, BLOC, NA, CH, CELLS)[:, :, :, 4, :]
        .transpose(0, 2, 1, 3)).reshape(NCORES, NA, 128, ROW // 128)
    # object-major gt: partition p=(b%4)*32+o, col j2=b//4
    gtb = np.asarray(gt_boxes, dtype=np.float32) \
        .reshape(NCORES, J2, 4, O, 4).transpose(0, 2, 3, 1, 4) \
        .reshape(NCORES, 128, J2 * 4)
    gtb = np.ascontiguousarray(gtb)
    # wrapped x/y copies for on-chip gather-index computation:
    # value at [16g+q, h*256+s] = coord of object (b = h*128 + s//2,
    # o = 16*(s%2) + q), replicated across the 8 16-partition groups
    gb = np.asarray(gt_boxes, dtype=np.float32).reshape(NCORES, BLOC, O, 4)
    col = np.arange(512)
    b_of = (col // 256) * 128 + (col % 256) // 2          # [512]
    q = np.arange(16)
    o_of = 16 * (col % 2)[None, :] + q[:, None]           # [16, 512]
    gtw = np.empty((NCORES, 128, 1024), dtype=np.float32)
    for c in range(2):
        w16 = gb[:, b_of[None, :], o_of, c]               # [NCORES, 16, 512]
        gtw[:, :, c * 512:(c + 1) * 512] = np.tile(w16, (1, 8, 1))
    clsf = np.asarray(gt_class).astype(np.float32) \
        .reshape(NCORES, J2, 4, O).transpose(0, 2, 3, 1) \
        .reshape(NCORES, 128, J2)
    clsf = np.ascontiguousarray(clsf)
    return det_g, conf, gtb, gtw, clsf


def _reduce_partials(P):
    """P: [ncores, 128, 32] fp32 partials -> the 4 scalar losses."""
    S = P.astype(np.float64).sum(axis=(0, 1))
    T = S[0:20].reshape(4, 5).sum(axis=0)
    coord, confobj, confsub, clsq, wsum = T
    dense = S[20:25].sum()
    obj_loss = 5.0 * coord + confobj
    no_obj_loss = 0.5 * (dense - confsub)
    conf_loss = clsq + wsum
    loss = obj_loss + no_obj_loss + conf_loss
    return (np.float32(loss), np.float32(obj_loss),
            np.float32(no_obj_loss), np.float32(conf_loss))


LAST_RESULT = None


def kernel(detection_result, gt_boxes, gt_class):
    import os
    from concourse.bass_utils import run_bass_kernel_spmd

    nc, consts = _get_built()
    det_g, conf, gtb, gtw, clsf = _prep_inputs(detection_result, gt_boxes,
                                               gt_class)

    in_maps = []
    for c in range(NCORES):
        m = {"detg": det_g[c], "conf": conf[c], "gtb": gtb[c],
             "gtw": gtw[c], "clsf": clsf[c]}
        m.update(consts)
        in_maps.append(m)

    kw = {}
    if os.environ.get("DETLOSS_TRACE"):
        kw["trace"] = True
        td = os.environ.get("DETLOSS_TRACE_DIR")
        if td:
            os.makedirs(td, exist_ok=True)
            kw["tmpdir"] = td
    res = run_bass_kernel_spmd(nc, in_maps, core_ids=list(range(NCORES)), **kw)
    global LAST_RESULT
    LAST_RESULT = res
    P = np.stack([res.results[c]["out"] for c in range(NCORES)])
    return _reduce_partials(P)


# revision 69
# speedup vs baseline: 1.1818x; 1.0086x over previous
"""Trainium2 Bass kernel for nn_DetectionLoss (YOLO-style detection loss).

Strategy (pure data parallel over 8 NeuronCores, 256 images each):
  - Host relayouts det to row-per-cell [img*169+cell, 128ch] bf16 (125 ch
    + 3 zero pad). The object gather is then two GPSIMD dma_gather calls
    (4096 indexed 256B-row fetches each) whose output lands DIRECTLY in
    the object-major [p=(b%4)*32+o, j2, ch] layout - no on-chip
    transposes, and the device reads only ~3 MB instead of 22 MB.
  - Gather indices (img*169 + cell of each object) are computed on-chip
    from gt_boxes (floor/k chain + PE partition-shuffle into the wrapped
    idx layout).
  - DVE does IoU / argmax / last-writer-wins dedup / loss assembly in 2
    passes (one per gather half); dedup runs image-per-partition
    ([128, 32*32] ops); class terms run in bf16 2x mode.
  - Dense no-obj conf sum reads a host-extracted [5, 43264] f32 copy of
    the conf channels (contiguous rows).
  - Output: per-core partial sums [128, 16]; host reduces across cores.
"""
import numpy as np

GRID = 13
NA = 5
NCLS = 20
CH = 25
NCH = NA * CH          # 125
CELLS = GRID * GRID    # 169
O = 32                 # objects per image
B = 2048               # global batch
NCORES = 8
BLOC = B // NCORES     # 256 images per core
ROW = BLOC * CELLS     # 43264 cells per core
NOBJ = BLOC * O        # 8192 objects per core
J2 = NOBJ // 128       # 64 object columns
NPASS = 2              # one pass per gather half (128 images each)
JPP = J2 // NPASS      # 32

ANCHORS = np.array([1.3221, 1.73145, 3.19275, 4.00944, 5.05587,
                    8.09892, 9.47112, 4.84053, 11.2364, 10.0071],
                   dtype=np.float32)

_CACHE = {}


def _make_consts():
    """Host-precomputed, data-independent constant input tensors."""
    consts = {}
    consts["c_ident"] = np.eye(128, dtype=np.float32)
    consts["c_iota5"] = np.tile(np.arange(5, dtype=np.float32), (128, 1))
    consts["c_iota5m"] = np.tile(np.arange(5, dtype=np.float32) - 99.0, (128, 1))
    consts["c_iota20"] = np.tile(np.arange(NCLS, dtype=np.float32), (128, 1))
    # anchor w/h prescale for permuted columns 10..20 = (r=2|3, a): [s2*5, s3*5]
    s23 = np.concatenate([ANCHORS[0::2] / GRID, ANCHORS[1::2] / GRID]) \
        .astype(np.float32)
    consts["c_s23"] = np.tile(s23, (128, 1))
    # strict upper-triangular pair mask over (o, o2): 1.0 iff o2 > o
    tri = (np.arange(O)[None, :] > np.arange(O)[:, None]).astype(np.float32)
    consts["c_tri"] = np.tile(tri.reshape(1, O * O), (128, 1))
    # rowbase for the wrapped idx layout: rbw[p, h*256+s] = 169*(s//2)
    col = np.arange(512)
    consts["c_rbw"] = np.tile(
        (CELLS * ((col % 256) // 2)).astype(np.float32), (128, 1))
    return consts


def _build():
    """Build the Bass module (emitted once, cached)."""
    import concourse.bacc as bacc
    import concourse.tile as tile
    from concourse import mybir

    f32 = mybir.dt.float32
    bf16 = mybir.dt.bfloat16
    i16 = mybir.dt.int16
    ALU = mybir.AluOpType
    AX = mybir.AxisListType
    ACT = mybir.ActivationFunctionType

    nc = bacc.Bacc(None, target_bir_lowering=False, debug=False)

    detg = nc.dram_tensor("detg", [ROW, 128], bf16, kind="ExternalInput")
    conf = nc.dram_tensor("conf", [NA, 128, ROW // 128], f32,
                          kind="ExternalInput")
    gtb = nc.dram_tensor("gtb", [128, J2 * 4], f32, kind="ExternalInput")
    gtw = nc.dram_tensor("gtw", [128, 1024], f32, kind="ExternalInput")
    clsf = nc.dram_tensor("clsf", [128, J2], f32, kind="ExternalInput")
    c_ident = nc.dram_tensor("c_ident", [128, 128], f32, kind="ExternalInput")
    c_iota5 = nc.dram_tensor("c_iota5", [128, 5], f32, kind="ExternalInput")
    c_iota5m = nc.dram_tensor("c_iota5m", [128, 5], f32, kind="ExternalInput")
    c_iota20 = nc.dram_tensor("c_iota20", [128, NCLS], f32,
                              kind="ExternalInput")
    c_s23 = nc.dram_tensor("c_s23", [128, 2 * NA], f32, kind="ExternalInput")
    c_tri = nc.dram_tensor("c_tri", [128, O * O], f32, kind="ExternalInput")
    c_rbw = nc.dram_tensor("c_rbw", [128, 512], f32, kind="ExternalInput")
    out = nc.dram_tensor("out", [128, 32], f32, kind="ExternalOutput")

    with tile.TileContext(nc) as tc:
        with tc.tile_pool(name="cpool", bufs=1) as cp, \
             tc.tile_pool(name="work", bufs=1) as wk, \
             tc.tile_pool(name="psA", bufs=2, space="PSUM") as psA:

            # ---- small loads (gtw + rowbase first: idx critical path) ----
            t_gtw = wk.tile([128, 1024], f32)
            nc.sync.dma_start(t_gtw[:], gtw[:])
            t_rbw = cp.tile([128, 512], f32)
            nc.sync.dma_start(t_rbw[:], c_rbw[:])
            t_gtb = wk.tile([128, J2 * 4], f32)
            nc.sync.dma_start(t_gtb[:], gtb[:])
            t_cls = wk.tile([128, J2], f32)
            nc.sync.dma_start(t_cls[:], clsf[:])
            t_id = cp.tile([128, 128], f32)
            t_i5 = cp.tile([128, 5], f32)
            t_i5m = cp.tile([128, 5], f32)
            t_i20 = cp.tile([128, NCLS], f32)
            t_s23 = cp.tile([128, 2 * NA], f32)
            t_tri = cp.tile([128, O * O], f32)
            nc.sync.dma_start(t_id[:], c_ident[:])
            nc.sync.dma_start(t_i5[:], c_iota5[:])
            nc.sync.dma_start(t_i5m[:], c_iota5m[:])
            nc.sync.dma_start(t_i20[:], c_iota20[:])
            nc.sync.dma_start(t_s23[:], c_s23[:])
            nc.sync.dma_start(t_tri[:], c_tri[:])

            # ---- gather row indices, computed directly in the wrapped
            # layout from the host-wrapped x/y copies ----
            # computed per half so the first gather launches ASAP
            t_wfx = wk.tile([128, 512], f32)
            t_wfy = wk.tile([128, 512], f32)
            t_wgx = wk.tile([128, 512], f32)
            t_wgy = wk.tile([128, 512], f32)
            t_wi = wk.tile([128, 512], mybir.dt.int32)
            t_iw = []
            for h in range(2):
                hs = slice(h * 256, (h + 1) * 256)
                nc.vector.tensor_scalar_mul(t_wfx[:, hs], t_gtw[:, hs],
                                            float(GRID))
                nc.vector.tensor_scalar_mul(
                    t_wfy[:, hs], t_gtw[:, 512 + h * 256:768 + h * 256],
                    float(GRID))
                for t_f, t_g in ((t_wfx, t_wgx), (t_wfy, t_wgy)):
                    nc.vector.tensor_copy(t_wi[:, hs], t_f[:, hs])
                    nc.vector.tensor_copy(t_g[:, hs], t_wi[:, hs])
                    nc.vector.tensor_tensor(t_wi[:, hs].bitcast(f32),
                                            t_g[:, hs], t_f[:, hs], ALU.is_gt)
                    nc.vector.tensor_sub(t_g[:, hs], t_g[:, hs],
                                         t_wi[:, hs].bitcast(f32))
                nc.vector.scalar_tensor_tensor(
                    out=t_wfx[:, hs], in0=t_wgy[:, hs], scalar=float(GRID),
                    in1=t_wgx[:, hs], op0=ALU.mult, op1=ALU.add)
                nc.vector.tensor_add(t_wfx[:, hs], t_wfx[:, hs], t_rbw[:, hs])
                # own idx tile (dma_gather idx APs must stay < 512B offset)
                t_i = wk.tile([128, 256], i16, name=f"t_iw{h}")
                t_iw.append(t_i)
                nc.scalar.activation(t_i[:], t_wfx[:, hs], ACT.Copy)

            gv = t_gtb[:].rearrange("p (j c) -> p j c", c=4)
            x_ap = gv[:, :, 0]
            y_ap = gv[:, :, 1]
            w_ap = gv[:, :, 2]
            h_ap = gv[:, :, 3]

            # ---- cell coords (DVE, object-major [128, 64]) ----
            t_mx = wk.tile([128, J2], f32)
            t_my = wk.tile([128, J2], f32)
            t_tx = wk.tile([128, J2], f32)
            t_ty = wk.tile([128, J2], f32)
            t_gx = wk.tile([128, J2], f32)
            t_gy = wk.tile([128, J2], f32)
            t_k = wk.tile([128, J2], f32)
            t_scr0 = wk.tile([128, J2], f32)
            nc.vector.tensor_scalar_mul(t_mx[:], x_ap, float(GRID))
            nc.vector.tensor_scalar_mul(t_my[:], y_ap, float(GRID))
            # floor(v), robust to the fp->int rounding mode:
            #   i = cvt(v); fi = cvt_back(i); gx = fi - (fi > v)
            t_i32 = wk.tile([128, J2], mybir.dt.int32)
            for t_m_, t_g_ in ((t_mx, t_gx), (t_my, t_gy)):
                nc.vector.tensor_copy(t_i32[:], t_m_[:])
                nc.vector.tensor_copy(t_g_[:], t_i32[:])
                nc.vector.tensor_tensor(t_scr0[:], t_g_[:], t_m_[:], ALU.is_gt)
                nc.vector.tensor_sub(t_g_[:], t_g_[:], t_scr0[:])
            nc.vector.tensor_sub(t_tx[:], t_mx[:], t_gx[:])
            nc.vector.tensor_sub(t_ty[:], t_my[:], t_gy[:])
            nc.vector.scalar_tensor_tensor(
                out=t_k[:], in0=t_gy[:], scalar=float(GRID), in1=t_gx[:],
                op0=ALU.mult, op1=ALU.add)

            # ---- gt-side quantities hoisted out of the passes ----
            t_hw2 = wk.tile([128, J2], f32)
            t_hh2 = wk.tile([128, J2], f32)
            t_gx0 = wk.tile([128, J2], f32)
            t_gy0 = wk.tile([128, J2], f32)
            t_gx1 = wk.tile([128, J2], f32)
            t_gy1 = wk.tile([128, J2], f32)
            t_a1 = wk.tile([128, J2], f32)
            t_mm2 = wk.tile([128, J2], f32)
            nc.vector.tensor_scalar_mul(t_hw2[:], w_ap, 0.5)
            nc.vector.tensor_scalar_mul(t_hh2[:], h_ap, 0.5)
            nc.vector.tensor_sub(t_gx0[:], x_ap, t_hw2[:])
            nc.vector.tensor_add(t_gx1[:], x_ap, t_hw2[:])
            nc.vector.tensor_sub(t_gy0[:], y_ap, t_hh2[:])
            nc.vector.tensor_add(t_gy1[:], y_ap, t_hh2[:])
            # +1 folded in for the intersection-width computation
            nc.vector.tensor_scalar_add(t_gx1[:], t_gx1[:], 1.0)
            nc.vector.tensor_scalar_add(t_gy1[:], t_gy1[:], 1.0)
            nc.vector.tensor_scalar(t_a1[:], t_hw2[:], 2.0, 1.0,
                                    ALU.mult, ALU.add)
            nc.vector.tensor_scalar(t_mm2[:], t_hh2[:], 2.0, 1.0,
                                    ALU.mult, ALU.add)
            nc.vector.tensor_mul(t_a1[:], t_a1[:], t_mm2[:])
            # coord targets (tx, ty, w, h) interleaved, bf16
            t_txyz = wk.tile([128, J2 * 4], bf16)
            tv = t_txyz[:].rearrange("p (j c) -> p j c", c=4)
            nc.vector.tensor_copy(tv[:, :, 0], t_tx[:])
            nc.vector.tensor_copy(tv[:, :, 1], t_ty[:])
            nc.vector.tensor_copy(tv[:, :, 2], w_ap)
            nc.vector.tensor_copy(tv[:, :, 3], h_ap)

            # ---- the object gather: dma_gather (1024 objects each) ----
            # out[p, j2l, ch] = detg[idx[n = j2l*128 + p], :] with
            # n = (b%128)*32 + o -> p = (b%4)*32 + o, j2l = (b%128)//4
            # HW quirks: num_idxs >= 2048 crashes; idx AP byte offsets
            # must stay < 512 (two [128, 256] tiles -> offsets 0..384)
            t_GTq = []
            for q in range(NPASS):
                t_GT = wk.tile([128, JPP * 128], bf16, name=f"t_GT{q}")
                t_GTq.append(t_GT)
            for g in range(8):
                q, gq = g // 4, g % 4
                nc.gpsimd.dma_gather(
                    out_ap=t_GTq[q][:, gq * 1024:(gq + 1) * 1024]
                    .rearrange("p (j ch) -> p j ch", ch=128),
                    in_ap=detg[(g // 4) * (ROW // 2):(g // 4 + 1) * (ROW // 2)],
                    idxs_ap=t_iw[g // 4][:, (g % 4) * 64:(g % 4 + 1) * 64],
                    num_idxs=1024, num_idxs_reg=1024, elem_size=128)

            t_stage = wk.tile([128, 32], f32)
            nc.vector.memset(t_stage[:], 0.0)

            # ---- dense conf sum (independent of gathers; runs early) ----
            for a in range(NA):
                t_cf = wk.tile([128, ROW // 128], f32, tag="cf", bufs=2,
                               name=f"t_cf{a}")
                nc.sync.dma_start(t_cf[:], conf[a])
                t_cfs = wk.tile([128, ROW // 128], f32, tag="cfs", bufs=2,
                                name=f"t_cfs{a}")
                nc.vector.tensor_mul(t_cfs[:], t_cf[:], t_cf[:])
                nc.vector.reduce_sum(
                    t_stage[:, 20 + a:21 + a], t_cfs[:], axis=AX.X)

            # per-pass work tiles (reused across passes)
            def w5(nm):
                return wk.tile([128, JPP * NA], f32, name=nm)
            t_iou = w5("t_iou"); t_scr = w5("t_scr"); t_scr2 = w5("t_scr2")
            t_bx0 = w5("t_bx0"); t_by0 = w5("t_by0")
            t_bx1 = w5("t_bx1"); t_by1 = w5("t_by1")
            t_ix0 = w5("t_ix0"); t_iy0 = w5("t_iy0")
            t_inter = w5("t_inter"); t_den = w5("t_den")
            t_ohA = w5("t_ohA")

            def w1(nm):
                return wk.tile([128, JPP], f32, name=nm)
            t_mm = w1("t_mm")
            t_aidx = w1("t_aidx"); t_sid = w1("t_sid"); t_win = w1("t_win")
            t_sidm = wk.tile([128, O], f32)       # [p=bi*32+j2l, o]
            t_deadm = wk.tile([128, O], f32)      # [p=bi*32+j2l, o]
            nc.vector.memset(t_sidm[:], 0.0)
            t_eqp = wk.tile([128, O * O], f32)    # [p=(bi,j2l), o*o2]
            # class onehot for ALL j2 (pass-independent; runs pre-gather)
            t_oh = wk.tile([128, J2 * NCLS], bf16)
            nc.vector.tensor_tensor(
                t_oh[:].rearrange("p (j c) -> p j c", c=NCLS),
                t_cls[:].rearrange("p (j one) -> p j one", one=1)
                .to_broadcast([128, J2, NCLS]),
                t_i20[:].rearrange("p (one c) -> p one c", one=1)
                .to_broadcast([128, J2, NCLS]),
                ALU.is_equal)
            t_qc = wk.tile([128, JPP * NA * NCLS], bf16)
            t_half = wk.tile([128, JPP * NA * 10], bf16)
            t_qcls = wk.tile([128, JPP * NA], bf16)
            t_s2sb = wk.tile([128, JPP * NA], bf16)
            t_dv = wk.tile([128, JPP * NA * 4], bf16)
            # stacked masked terms: [p, term(5), j, a]
            t_terms = wk.tile([128, 5 * JPP * NA], f32)
            t_wm = wk.tile([128, 5 * JPP * NA], f32)

            def r5(t):
                return t[:].rearrange("p (j a) -> p j a", a=NA)

            def trm(i):  # term slice i as [p, j, a]
                return t_terms[:, i * JPP * NA:(i + 1) * JPP * NA] \
                    .rearrange("p (j a) -> p j a", a=NA)

            for ps in range(NPASS):
                t_GT = t_GTq[ps]
                t_GSQ = wk.tile([128, JPP * NA * NCLS], bf16,
                                name=f"t_GSQ{ps}", tag="gsq", bufs=2)
                gfull = t_GT[:].rearrange("p (j ch) -> p j ch", ch=128)
                # prescale columns 10..20 (= q2, q3 across anchors) in place
                nc.vector.tensor_tensor(
                    gfull[:, :, 10:20], gfull[:, :, 10:20],
                    t_s23[:].rearrange("p (one s) -> p one s", one=1)
                    .to_broadcast([128, JPP, 10]),
                    ALU.mult)
                # class squares only (conf squares fused into ACT ops below)
                nc.scalar.activation(
                    t_GSQ[:].rearrange("p (j c) -> p j c", c=NA * NCLS),
                    gfull[:, :, 25:125], ACT.Square)

                jsl = slice(ps * JPP, (ps + 1) * JPP)
                q0 = gfull[:, :, 0:5]
                q1 = gfull[:, :, 5:10]
                q2 = gfull[:, :, 10:15]   # pre-scaled by anchor w / GRID
                q3 = gfull[:, :, 15:20]   # pre-scaled by anchor h / GRID
                q4 = gfull[:, :, 20:25]
                qclsv = gfull[:, :, 25:125].rearrange(
                    "p j (a c) -> p j a c", c=NCLS)   # [p, j, a, 20] bf16
                qclssq = t_GSQ[:].rearrange(
                    "p (j a c) -> p j a c", a=NA, c=NCLS)

                def b5(ap2d):  # [128, JPP] -> broadcast [128, JPP, 5]
                    return ap2d.rearrange("p (j one) -> p j one", one=1) \
                               .to_broadcast([128, JPP, NA])

                def c5(tile1):  # const [128, 5] -> [128, JPP, 5]
                    return tile1[:].rearrange("p (one a) -> p one a", one=1) \
                                   .to_broadcast([128, JPP, NA])

                # ---- IoU (per object x anchor) ----
                # bx0 = (px+gx)/13 - pw/2 ; by0 = (py+gy)/13 - ph/2
                nc.vector.tensor_tensor(r5(t_bx0), q0, b5(t_gx[:, jsl]),
                                        ALU.add)
                nc.vector.tensor_scalar_mul(t_bx0[:], t_bx0[:], 1.0 / GRID)
                nc.vector.scalar_tensor_tensor(
                    out=r5(t_bx0), in0=q2, scalar=-0.5, in1=r5(t_bx0),
                    op0=ALU.mult, op1=ALU.add)
                nc.vector.tensor_tensor(r5(t_by0), q1, b5(t_gy[:, jsl]),
                                        ALU.add)
                nc.vector.tensor_scalar_mul(t_by0[:], t_by0[:], 1.0 / GRID)
                nc.vector.scalar_tensor_tensor(
                    out=r5(t_by0), in0=q3, scalar=-0.5, in1=r5(t_by0),
                    op0=ALU.mult, op1=ALU.add)
                # pw+1, ph+1 (used for both bx1+1/by1+1 and a2); the gt-side
                # +1s are folded into the hoisted gx1/gy1
                nc.vector.tensor_scalar(r5(t_scr), q2, 1.0, 1.0,
                                        ALU.mult, ALU.add)
                nc.vector.tensor_scalar(r5(t_scr2), q3, 1.0, 1.0,
                                        ALU.mult, ALU.add)
                nc.vector.tensor_tensor(r5(t_bx1), r5(t_bx0), r5(t_scr),
                                        ALU.add)
                nc.vector.tensor_tensor(r5(t_by1), r5(t_by0), r5(t_scr2),
                                        ALU.add)
                # intersection (+1 folded): ix1+1 = min(bx1+1, gx1+1)
                nc.vector.tensor_tensor(r5(t_ix0), r5(t_bx0), b5(t_gx0[:, jsl]),
                                        ALU.max)
                nc.vector.tensor_tensor(r5(t_iy0), r5(t_by0), b5(t_gy0[:, jsl]),
                                        ALU.max)
                nc.vector.tensor_tensor(r5(t_bx1), r5(t_bx1), b5(t_gx1[:, jsl]),
                                        ALU.min)
                nc.vector.tensor_tensor(r5(t_by1), r5(t_by1), b5(t_gy1[:, jsl]),
                                        ALU.min)
                nc.vector.tensor_sub(t_bx1[:], t_bx1[:], t_ix0[:])
                nc.vector.tensor_sub(t_by1[:], t_by1[:], t_iy0[:])
                nc.vector.tensor_mul(t_inter[:], t_bx1[:], t_by1[:])
                # a2 = (pw+1)*(ph+1); denom = a1 + a2 - inter
                nc.vector.tensor_mul(t_den[:], t_scr[:], t_scr2[:])
                nc.vector.tensor_tensor(r5(t_den), r5(t_den), b5(t_a1[:, jsl]),
                                        ALU.add)
                nc.vector.tensor_sub(t_den[:], t_den[:], t_inter[:])
                nc.vector.reciprocal(t_den[:], t_den[:])
                nc.vector.tensor_mul(t_iou[:], t_inter[:], t_den[:])

                # ---- argmax over anchors (first max wins) ----
                nc.vector.reduce_max(t_mm[:], r5(t_iou), axis=AX.X)
                nc.vector.tensor_tensor(
                    r5(t_scr), r5(t_iou), b5(t_mm), ALU.is_equal)
                nc.vector.tensor_tensor(
                    r5(t_scr2), r5(t_scr), c5(t_i5m), ALU.mult)
                nc.vector.tensor_reduce(
                    t_aidx[:], r5(t_scr2), axis=AX.X, op=ALU.min)
                nc.vector.tensor_scalar_add(t_aidx[:], t_aidx[:], 99.0)

                # ---- slot id s = 169*aidx + k ; last-writer-wins dedup ----
                # layout: object (b,o) at partition p=(bi=b%4)*32+o, col j2=b//4
                nc.vector.scalar_tensor_tensor(
                    out=t_sid[:], in0=t_aidx[:], scalar=float(CELLS),
                    in1=t_k[:, jsl], op0=ALU.mult, op1=ALU.add)
                # transpose -> [j2l, (bi,o)], then 4 ACT copies pack an
                # image per partition: sidm[p = bi*32 + j2l, o] (partition
                # starts must be 32-aligned, so 16 rows used per 32-block).
                # The coord/conf/class DVE blocks below are emitted between
                # the PE/ACT legs so their latency hides under DVE compute.
                t_tp1 = psA.tile([JPP, 128], f32, space="PSUM", tag="ded",
                                 bufs=1)
                nc.tensor.transpose(out=t_tp1[:], in_=t_sid[:],
                                    identity=t_id[:])
                for bi in range(4):
                    nc.scalar.activation(
                        t_sidm[bi * 32:bi * 32 + JPP, :],
                        t_tp1[:, bi * O:(bi + 1) * O], ACT.Copy)
                sma = t_sidm[:].rearrange("p (o one) -> p o one", one=1) \
                               .to_broadcast([128, O, O])
                smb = t_sidm[:].rearrange("p (one o2) -> p one o2", one=1) \
                               .to_broadcast([128, O, O])
                eqv = t_eqp[:].rearrange("p (o o2) -> p o o2", o2=O)
                nc.vector.tensor_tensor(eqv, sma, smb, ALU.is_equal)
                triv = t_tri[:].rearrange("p (o o2) -> p o o2", o2=O)
                nc.vector.tensor_tensor(eqv, eqv, triv, ALU.mult)
                nc.vector.tensor_reduce(
                    t_deadm[:].rearrange("p (o one) -> p o one", one=1),
                    eqv, axis=AX.X, op=ALU.max)
                t_tp2 = psA.tile([O, 128], f32, space="PSUM", tag="ded2",
                                 bufs=1)
                nc.tensor.transpose(out=t_tp2[:], in_=t_deadm[:],
                                    identity=t_id[:])
                for bi in range(4):
                    nc.scalar.activation(
                        t_win[bi * O:(bi + 1) * O, :],
                        t_tp2[:, bi * 32:bi * 32 + JPP], ACT.Copy)
                nc.vector.tensor_scalar(
                    t_win[:], t_win[:], -1.0, 1.0, ALU.mult, ALU.add)
                nc.vector.tensor_tensor(
                    r5(t_ohA), b5(t_aidx), c5(t_i5), ALU.is_equal)
                nc.vector.tensor_tensor(
                    trm(4), r5(t_ohA), b5(t_win), ALU.mult)

                # ---- coord SSE (-> term slice 0) ----
                # iterate (j, c, a) so the bf16 reads stay stride-1;
                # t_dv is stored [p, j, c, a]
                dvv = t_dv[:].rearrange("p (j c a) -> p j c a", c=4, a=NA)
                nc.vector.tensor_tensor(
                    dvv,
                    gfull[:, :, 0:20].rearrange("p j (c a) -> p j c a", a=NA),
                    t_txyz[:, ps * JPP * 4:(ps + 1) * JPP * 4]
                    .rearrange("p (j c one) -> p j c one", one=1, c=4)
                    .to_broadcast([128, JPP, 4, NA]),
                    ALU.subtract)
                nc.vector.tensor_mul(t_dv[:], t_dv[:], t_dv[:])
                # sum over c with 3 contiguous adds (avoids a strided reduce)
                nc.vector.tensor_tensor(
                    trm(0), dvv[:, :, 0, :], dvv[:, :, 1, :], ALU.add)
                nc.vector.tensor_tensor(
                    r5(t_scr2), dvv[:, :, 2, :], dvv[:, :, 3, :], ALU.add)
                nc.vector.tensor_tensor(
                    trm(0), trm(0), r5(t_scr2), ALU.add)

                # ---- conf terms: (1-q4)^2 -> slice 1, q4^2 -> slice 2 ----
                nc.vector.tensor_scalar(
                    r5(t_scr), q4, -1.0, 1.0, ALU.mult, ALU.add)
                nc.vector.tensor_mul(trm(1), r5(t_scr), r5(t_scr))
                nc.vector.tensor_tensor(trm(2), q4, q4, ALU.mult)

                # ---- class terms (bf16 2x) -> slice 3 ----
                qcv = t_qc[:].rearrange("p (j a c) -> p j a c", a=NA, c=NCLS)
                nc.vector.tensor_tensor(
                    qcv, qclsv,
                    t_oh[:, ps * JPP * NCLS:(ps + 1) * JPP * NCLS]
                    .rearrange("p (j one c) -> p j one c", one=1, c=NCLS)
                    .to_broadcast([128, JPP, NA, NCLS]),
                    ALU.mult)
                # halve the 20-wide reduces with one bf16-2x add first
                hv = t_half[:].rearrange("p (j a c) -> p j a c", a=NA, c=10)
                qcv4 = t_qc[:].rearrange("p (j a h c) -> p j a h c",
                                         a=NA, h=2, c=10)
                sqv4 = t_GSQ[:].rearrange("p (j a h c) -> p j a h c",
                                          a=NA, h=2, c=10)
                with nc.allow_low_precision(reason="20-elem sums, fp32 "
                                            "internal, 2e-2 tolerance"):
                    nc.vector.tensor_tensor(
                        hv, qcv4[:, :, :, 0, :], qcv4[:, :, :, 1, :], ALU.add)
                    nc.vector.tensor_reduce(
                        r5(t_qcls), hv, axis=AX.X, op=ALU.add)
                    nc.vector.tensor_tensor(
                        hv, sqv4[:, :, :, 0, :], sqv4[:, :, :, 1, :], ALU.add)
                    nc.vector.tensor_reduce(
                        r5(t_s2sb), hv, axis=AX.X, op=ALU.add)
                # cls_t = S2 - 2*qcls  (the +1 handled via sum(W))
                nc.vector.scalar_tensor_tensor(
                    out=trm(3), in0=r5(t_qcls), scalar=-2.0, in1=r5(t_s2sb),
                    op0=ALU.mult, op1=ALU.add)

                # ---- masked accumulate: 2 ops for all 5 terms ----
                nc.vector.tensor_tensor(
                    t_wm[:].rearrange("p (t ja) -> p t ja", t=5),
                    t_terms[:].rearrange("p (t ja) -> p t ja", t=5),
                    t_terms[:, 4 * JPP * NA:5 * JPP * NA]
                    .rearrange("p (one ja) -> p one ja", one=1)
                    .to_broadcast([128, 5, JPP * NA]),
                    ALU.mult)
                nc.vector.tensor_reduce(
                    t_stage[:, ps * 5:(ps + 1) * 5]
                    .rearrange("p (t one) -> p t one", one=1),
                    t_wm[:].rearrange("p (t ja) -> p t ja", t=5),
                    axis=AX.X, op=ALU.add)

            nc.sync.dma_start(out[:], t_stage[:])

    nc.compile()
    return nc


def _get_built():
    if "nc" not in _CACHE:
        _CACHE["nc"] = _build()
        _CACHE["consts"] = _make_consts()
    return _CACHE["nc"], _CACHE["consts"]


def _prep_inputs(detection_result, gt_boxes, gt_class):
    """Host-side layout marshalling (data-independent reshapes only)."""
    import ml_dtypes
    det = np.asarray(detection_result, dtype=np.float32)
    # row-per-cell bf16: [core][img*169+cell][128ch'], where the channel
    # columns are permuted so every per-quantity view is contiguous:
    # ch' = r*5 + a for coord/conf r<5, then 25 + a*20 + c for classes
    perm = np.empty(NCH, dtype=np.int64)
    for r in range(5):
        for a in range(NA):
            perm[r * 5 + a] = a * CH + r
    for a in range(NA):
        for c in range(NCLS):
            perm[25 + a * NCLS + c] = a * CH + 5 + c
    det_g = np.zeros((B, CELLS, 128), dtype=ml_dtypes.bfloat16)
    det_g[:, :, :NCH] = det.reshape(B, NCH, CELLS)[:, perm].transpose(0, 2, 1)
    det_g = det_g.reshape(NCORES, ROW, 128)
    # dense copy of the 5 conf channels: [core][anchor][img][cell] f32
    conf = np.ascontiguousarray(
        det.reshape(NCORES, BLOC, NA, CH, CELLS)[:, :, :, 4, :]
        .transpose(0, 2, 1, 3)).reshape(NCORES, NA, 128, ROW // 128)
    # object-major gt: partition p=(b%4)*32+o, col j2=b//4
    gtb = np.asarray(gt_boxes, dtype=np.float32) \
        .reshape(NCORES, J2, 4, O, 4).transpose(0, 2, 3, 1, 4) \
        .reshape(NCORES, 128, J2 * 4)
    gtb = np.ascontiguousarray(gtb)
    # wrapped x/y copies for on-chip gather-index computation:
    # value at [16g+q, h*256+s] = coord of object (b = h*128 + s//2,
    # o = 16*(s%2) + q), replicated across the 8 16-partition groups
    gb = np.asarray(gt_boxes, dtype=np.float32).reshape(NCORES, BLOC, O, 4)
    col = np.arange(512)
    b_of = (col // 256) * 128 + (col % 256) // 2          # [512]
    q = np.arange(16)
    o_of = 16 * (col % 2)[None, :] + q[:, None]           # [16, 512]
    gtw = np.empty((NCORES, 128, 1024), dtype=np.float32)
    for c in range(2):
        w16 = gb[:, b_of[None, :], o_of, c]               # [NCORES, 16, 512]
        gtw[:, :, c * 512:(c + 1) * 512] = np.tile(w16, (1, 8, 1))
    clsf = np.asarray(gt_class).astype(np.float32) \
        .reshape(NCORES, J2, 4, O).transpose(0, 2, 3, 1) \
        .reshape(NCORES, 128, J2)
    clsf = np.ascontiguousarray(clsf)
    return det_g, conf, gtb, gtw, clsf


def _reduce_partials(P):
    """P: [ncores, 128, 32] fp32 partials -> the 4 scalar losses."""
    S = P.astype(np.float64).sum(axis=(0, 1))
    T = S[0:20].reshape(4, 5).sum(axis=0)
    coord, confobj, confsub, clsq, wsum = T
    dense = S[20:25].sum()
    obj_loss = 5.0 * coord + confobj
    no_obj_loss = 0.5 * (dense - confsub)
    conf_loss = clsq + wsum
    loss = obj_loss + no_obj_loss + conf_loss
    return (np.float32(loss), np.float32(obj_loss),
            np.float32(no_obj_loss), np.float32(conf_loss))


LAST_RESULT = None


def kernel(detection_result, gt_boxes, gt_class):
    import os
    from concourse.bass_utils import run_bass_kernel_spmd

    nc, consts = _get_built()
    det_g, conf, gtb, gtw, clsf = _prep_inputs(detection_result, gt_boxes,
                                               gt_class)

    in_maps = []
    for c in range(NCORES):
        m = {"detg": det_g[c], "conf": conf[c], "gtb": gtb[c],
             "gtw": gtw[c], "clsf": clsf[c]}
        m.update(consts)
        in_maps.append(m)

    kw = {}
    if os.environ.get("DETLOSS_TRACE"):
        kw["trace"] = True
        td = os.environ.get("DETLOSS_TRACE_DIR")
        if td:
            os.makedirs(td, exist_ok=True)
            kw["tmpdir"] = td
    res = run_bass_kernel_spmd(nc, in_maps, core_ids=list(range(NCORES)), **kw)
    global LAST_RESULT
    LAST_RESULT = res
    P = np.stack([res.results[c]["out"] for c in range(NCORES)])
    return _reduce_partials(P)
